# revision 6
# baseline (speedup 1.0000x reference)
"""Trainium2 Bass kernel for CrossInferBlock (spatial+temporal cross attention
+ out-projection + residual + BatchNorm over (B,T,N)).

Sharding: data-parallel over B across 8 NeuronCores (one batch element per
core). BN batch statistics are combined via an 8-core AllGather (8KB ->
64KB) + a local DVE reduce -- measured ~4x faster than AllReduce on this
fabric (AllReduce has a ~32us fixed cost; AllGather ~a third of that).

Precision plan (the residual dominates the output; the attention branch is
~17% of output magnitude, so fp8 there is cheap in accuracy; measured
rel err 1.6e-2 vs the fp32 reference, gate 2e-2):
  - theta + g projections: fp8e4 DoubleRow (2 K-tiles/pass = 2x PE
    throughput); x/Wt/Wg uploaded e4m3 (weights x16), outputs stored e3m4.
  - phi projection: bf16 (phi/theta errors multiply in the attention
    scores, so one of the pair stays high-precision).
  - attention scores tw/sw -> e3m4 (x1/32); applies tp/sp run fp8 at the
    bf16 rate; stT stored e3m4 (x8, max |stT| = 11.5 < 15.5).
  - out-projection: e3m4 x e3m4 (Ww x32); PSUM descaled by 2^-8 at the
    bf16 residual add. BN stats/apply in fp32.
All scale factors are powers of two (exact).

Device-side token order is ACTOR-MAJOR: tok = j*T + t. stT however is
stored T-MAJOR (free = lc*TOK + t*128 + j): the 16 per-timestep spatial
read-modify-write adds (phase 2, on the congested DVE) become contiguous
(~0.7us vs 2.6us strided), while the 16 temporal init writes eat the
stride on the half-idle ACT engine in phase 1. The out-projection reads
stT through a strided moving AP (stride-128 over t), which the PE walks
at full rate, so PSUM/residual/output stay token-major.

Phase order is chosen for DMA just-in-time: g_act+theta (needs only
wg/wt/xf8 = 3MB) start ~13us in while phi's inputs (wp/xbf = 5MB) and
the phase-2/3 tensors (xf8b, ww) stream in behind. theta and phi run
lc-major against 4 (2 for phi) concurrent PSUM accumulation groups so
each stationary weight tile is loaded once, not once per token chunk
(saves ~20k LDWEIGHTS columns). g_sp is precomputed in phase 1c so
phase 2 is only sw/sp + the cheap contiguous RMW.

Collectives: one warm-up AllGather at load time absorbs the CC stream's
one-time ~26us setup; a second keyed on phase-1c data keeps the ring
recent. The real stats AllGather fires as soon as the last out-projection
epilogue lands. BN apply+store is split across the DVE and ACT engines
with bf16 stores on three HWDGE rings (the host upcasts to fp32).
"""

import sys

if "/opt/trn_rl_repo" not in sys.path:
    sys.path.insert(0, "/opt/trn_rl_repo")

import numpy as np
import ml_dtypes

import concourse.bass as bass
import concourse.bacc as bacc
import concourse.tile as tile
import concourse.mybir as mybir
from concourse.bass_utils import run_bass_kernel_spmd
from contextlib import ExitStack

F32 = mybir.dt.float32
BF16 = mybir.dt.bfloat16
F8E4 = mybir.dt.float8e4     # e4m3: DoubleRow-capable
F8E3 = mybir.dt.float8e3     # e3m4: 2x mantissa, bf16-rate matmuls
AX = mybir.AxisListType
OP = mybir.AluOpType
ACT_FN = mybir.ActivationFunctionType
DR = mybir.MatmulPerfMode.DoubleRow

N_CORES = 8
B, T, N, C = 8, 16, 128, 1024
L = C // 2            # 512
TOK = T * N           # 2048 tokens per batch element
NTOK_GLOBAL = B * T * N
JG = 8                # actors per temporal group
NGRP = N // JG        # 16 groups
BN_EPS = 1e-5

WG_SCALE = 16.0       # Wg uploaded x16 (e4m3)
WW_SCALE = 32.0       # Ww uploaded x32 (e3m4: normal range starts at 0.25)
G_DESCALE = 1.0 / WG_SCALE
SB_SCALE = 1.0 / 32.0            # attention scores into e3m4 (std ~1.4)
STT_SCALE = 8.0                  # stT e3m4 boost (max|stT|=11.5 < 15.5)
SP_SCALE = STT_SCALE / (N * (T + N)) / SB_SCALE    # 0.027778
TP_SCALE = STT_SCALE / (T * (T + N)) / SB_SCALE    # 0.222
OUT_DESCALE = 1.0 / (STT_SCALE * WW_SCALE)         # 2^-9

NCC = C // 128     # 8 c-chunks
NLC = L // 128     # 4 l-chunks
NCP = NCC // 2     # 4 c-chunk pairs (DoubleRow)
NTC = TOK // 512   # 4 token chunks

_compiled = None
_last_results = None

USE_COLLECTIVE = True


def ts(i, size):
    return bass.ts(i, size)


def _build():
    nc = bacc.Bacc("TRN2", target_bir_lowering=False, debug=False,
                   num_devices=N_CORES)

    # ---- DRAM I/O (token order: actor-major, tok = j*T + t) ----
    # inputs are pre-shuffled on the host into the SBUF tile layout
    # [128, chunk*free] so every load is a full-row (4-32KB/row) DMA
    xf8_d = nc.dram_tensor("xf8", [128, NCC * TOK], F8E4,
                           kind="ExternalInput")
    # second fp8 x copy in channel-major layout: the spatial projection's
    # stride-T token gather needs (a, tok) order, which would make the
    # token-chunk-major tile a 4-free-dim DoubleRow weights AP
    xf8b_d = nc.dram_tensor("xf8b", [128, NCC * TOK], F8E4,
                            kind="ExternalInput")
    xbf_d = nc.dram_tensor("xbf", [128, NCC * TOK], BF16,
                           kind="ExternalInput")
    wt_d = nc.dram_tensor("wt", [128, NCC * L], F8E4, kind="ExternalInput")
    wp_d = nc.dram_tensor("wp", [128, NCC * L], BF16, kind="ExternalInput")
    wg_d = nc.dram_tensor("wg", [128, NCC * L], F8E4, kind="ExternalInput")
    ww_d = nc.dram_tensor("ww", [128, NLC * C], F8E3, kind="ExternalInput")
    mask_d = nc.dram_tensor("mask", [128, 128], BF16, kind="ExternalInput")
    gb_d = nc.dram_tensor("gb", [128, 16], F32, kind="ExternalInput")
    outy_d = nc.dram_tensor("outy", [C, TOK], BF16, kind="ExternalOutput")

    with tile.TileContext(nc) as tc:
        with ExitStack() as outer:
            # ---------------- persistent pools ----------------
            cpool = outer.enter_context(tc.tile_pool(name="consts", bufs=1))
            wwpool = outer.enter_context(tc.tile_pool(name="wwp", bufs=1))
            stpool = outer.enter_context(tc.tile_pool(name="stp", bufs=1))
            statpool = outer.enter_context(tc.tile_pool(name="stats", bufs=1))
            pbig = outer.enter_context(
                tc.tile_pool(name="pbig", bufs=1, space="PSUM"))
            psmall = outer.enter_context(
                tc.tile_pool(name="psmall", bufs=1, space="PSUM"))
            drampool = outer.enter_context(
                tc.tile_pool(name="dramp", bufs=1, space="DRAM"))
            xbpool = outer.enter_context(tc.tile_pool(name="xbp", bufs=1))

            mask_sb = cpool.tile([128, 128], BF16, name="mask_sb",
                                 tag="mask_sb")
            gb_sb = cpool.tile([128, 16], F32, name="gb_sb", tag="gb_sb")
            ww_all = wwpool.tile([128, NLC * C], F8E3, name="ww_all", tag="ww")
            # stT is T-MAJOR: free = lc*TOK + t*128 + j
            stT = stpool.tile([128, NLC * TOK], F8E3, name="stT", tag="stT")

            stat_sum = statpool.tile([128, 32], F32, name="stat_sum",
                                     tag="stat_sum")
            stat_sq = statpool.tile([128, 32], F32, name="stat_sq",
                                    tag="stat_sq")
            red_in = statpool.tile([128, 16], F32, name="red_in", tag="red_in")
            ag_sb = statpool.tile([128, N_CORES * 16], F32, name="ag_sb",
                                  tag="ag_sb")
            red_out = statpool.tile([128, 16], F32, name="red_out",
                                    tag="red_out")
            scalev = statpool.tile([128, 8], F32, name="scalev", tag="scalev")
            biasv = statpool.tile([128, 8], F32, name="biasv", tag="biasv")

            cc_big_in = drampool.tile([128, 16], F32, name="cc_big_in",
                                      tag="cc_big_in")
            cc_big_out = drampool.tile([N_CORES * 128, 16], F32,
                                       name="cc_big_out", tag="cc_big_out")
            cc_warm_in = drampool.tile([128, 1], F32, name="cc_warm_in",
                                       tag="cc_warm_in")
            cc_warm_out = drampool.tile([N_CORES * 128, 1], F32,
                                        name="cc_warm_out", tag="cc_warm_out")
            cc_w2_in = drampool.tile([128, 1], F32, name="cc_w2_in",
                                     tag="cc_w2_in")
            cc_w2_out = drampool.tile([N_CORES * 128, 1], F32,
                                      name="cc_w2_out", tag="cc_w2_out")

            with ExitStack() as mid:
                thpool = mid.enter_context(tc.tile_pool(name="thp", bufs=1))
                gpool = mid.enter_context(tc.tile_pool(name="gp", bufs=1))
                attnpool = mid.enter_context(tc.tile_pool(name="attn", bufs=1))

                thT = thpool.tile([128, NLC * TOK], F8E3, name="thT",
                                  tag="thT")
                phT = thpool.tile([128, NLC * TOK], F8E3, name="phT",
                                  tag="phT")
                g_sp = [gpool.tile([128, L], F8E3, name=f"gsp{i}",
                                   tag=f"gsp{i}") for i in range(T)]
                g_act = [gpool.tile([128, L], F8E3, name=f"gact{j}",
                                    tag=f"gact{j}") for j in range(NGRP)]

                with ExitStack() as phase_a:
                    wpool = phase_a.enter_context(
                        tc.tile_pool(name="wp", bufs=1))

                    xf8 = xbpool.tile([128, NCC * TOK], F8E4, name="xf8",
                                      tag="xf8")
                    xf8b = xbpool.tile([128, NCC * TOK], F8E4, name="xf8b",
                                       tag="xf8b")
                    xbf = xbpool.tile([128, NCC * TOK], BF16, name="xbf",
                                      tag="xbf")
                    wt_all = wpool.tile([128, NCC * L], F8E4, name="wt_all",
                                        tag="wt")
                    wp_all = wpool.tile([128, NCC * L], BF16, name="wp_all",
                                        tag="wp")
                    wg_all = wpool.tile([128, NCC * L], F8E4, name="wg_all",
                                        tag="wg")

                    # input DMA schedule, critical-path first:
                    # the first compute (g_act) needs only wg + xf8 chunk 0;
                    # theta then needs wt + the rest of xf8; phi (after
                    # theta, ~25us later) needs wp + xbf; xf8b (phase 1c)
                    # and ww (phase 3) ride last on the slow ring.
                    CHW = NCC * 512          # flat cols per token chunk
                    nc.gpsimd.dma_start(wg_all[:], wg_d[:])
                    nc.sync.dma_start(xf8[:, 0:CHW], xf8_d[:, 0:CHW])
                    nc.gpsimd.dma_start(wt_all[:], wt_d[:])
                    nc.scalar.dma_start(xf8[:, ts(1, CHW)],
                                        xf8_d[:, ts(1, CHW)])
                    for tk in range(2, NTC):
                        nc.gpsimd.dma_start(xf8[:, ts(tk, CHW)],
                                            xf8_d[:, ts(tk, CHW)])
                    nc.sync.dma_start(wp_all[:], wp_d[:])
                    for tk in range(2):
                        nc.sync.dma_start(xbf[:, ts(tk, CHW)],
                                          xbf_d[:, ts(tk, CHW)])
                    nc.gpsimd.dma_start(mask_sb[:], mask_d[:])
                    nc.gpsimd.dma_start(gb_sb[:], gb_d[:])
                    for tk in range(2, NTC):
                        nc.scalar.dma_start(xbf[:, ts(tk, CHW)],
                                            xbf_d[:, ts(tk, CHW)])
                    nc.scalar.dma_start(xf8b[:], xf8b_d[:])
                    nc.scalar.dma_start(ww_all[:], ww_d[:])
                    if USE_COLLECTIVE:
                        # warm-up collective #1: pays the CC stream's
                        # one-time setup during the DMA load
                        nc.gpsimd.dma_start(cc_warm_in[:], gb_d[:, 0:1])
                        nc.gpsimd.collective_compute(
                            "AllGather", OP.bypass,
                            replica_groups=[list(range(N_CORES))],
                            ins=[cc_warm_in.opt()], outs=[cc_warm_out.opt()])

                    # views (x tiles are token-chunk-major: (tck, a, k))
                    xv8 = xf8.rearrange("p (tk a k) -> p tk a k",
                                        tk=NTC, a=NCC)
                    # spatial: tok = j*T + t (channel-major copy)
                    xsp8 = xf8b.rearrange("p (a j t) -> p a t j",
                                          a=NCC, t=T)
                    xbv = xbf.rearrange("p (tk a k) -> p tk a k",
                                        tk=NTC, a=NCC)
                    wgv = wg_all.rearrange("p (a l) -> p a l", a=NCC)
                    wtv = wt_all.rearrange("p (a l) -> p a l", a=NCC)

                    def xsl(c, tck):
                        return xbv[:, tck, c, :]

                    def wsl(w, c, lc):
                        return w[:, c * L + lc * 128:c * L + (lc + 1) * 128]

                    # ------- phase 1a: g_act (xf8+wg) then theta (wt) ------
                    for jg in range(NGRP):
                        tck = jg // 4
                        ps = pbig.tile([128, 512], F32, name="ps_ga",
                                       tag="ps_big", bufs=4)
                        for cp in range(NCP):
                            nc.tensor.matmul(
                                ps[:],
                                xv8[:, tck, 2 * cp:2 * cp + 2,
                                    ts(jg - 4 * tck, 128)],
                                wgv[:, 2 * cp:2 * cp + 2, :],
                                start=(cp == 0), stop=(cp == NCP - 1),
                                perf_mode=DR)
                        nc.scalar.mul(g_act[jg][:], ps[:], G_DESCALE)

                    # theta: fp8 DoubleRow, lc-major with 4 concurrent tck
                    # PSUM groups -- each wt tile is loaded once, serving
                    # 4 back-to-back matmuls (consecutive loads dedupe)
                    for lc in range(NLC):
                        pss = [pbig.tile([128, 512], F32, name=f"ps_th{t}",
                                         tag="ps_big", bufs=4)
                               for t in range(NTC)]
                        for cp in range(NCP):
                            for tck in range(NTC):
                                nc.tensor.matmul(
                                    pss[tck][:],
                                    wtv[:, 2 * cp:2 * cp + 2, ts(lc, 128)],
                                    xv8[:, tck, 2 * cp:2 * cp + 2, :],
                                    start=(cp == 0), stop=(cp == NCP - 1),
                                    perf_mode=DR)
                        for tck in range(NTC):
                            dst = thT[:, lc * TOK + tck * 512:
                                      lc * TOK + tck * 512 + 512]
                            nc.vector.tensor_scalar_mul(dst, pss[tck][:],
                                                        G_DESCALE)

                    # ------- phase 1b: phi (bf16; wp + xbf) ---------------
                    # lc-major over tck pairs (2 concurrent PSUM groups) so
                    # phi can start once xbf chunks 0-1 have landed
                    for half in range(2):
                        tcks = (2 * half, 2 * half + 1)
                        for lc in range(NLC):
                            pss = [pbig.tile([128, 512], F32,
                                             name=f"ps_ph{t}",
                                             tag="ps_big", bufs=4)
                                   for t in tcks]
                            for c in range(NCC):
                                for k, tck in enumerate(tcks):
                                    nc.tensor.matmul(
                                        pss[k][:], wsl(wp_all, c, lc),
                                        xsl(c, tck),
                                        start=(c == 0), stop=(c == NCC - 1))
                            for k, tck in enumerate(tcks):
                                dst = phT[:, lc * TOK + tck * 512:
                                          lc * TOK + tck * 512 + 512]
                                nc.vector.tensor_copy(dst, pss[k][:])

                    # ---- phase 1c: temporal attention + g_sp precompute --
                    # temporal INITIALIZES stT (t-major, strided write on the
                    # half-idle ACT engine); g_sp precomputed here so phase 2
                    # is only sw/sp
                    pend_tp = []   # (jg, twp)

                    def emit_tw(jg):
                        twp = psmall.tile([128, 128], F32, name="ps_tw",
                                          tag="ps_small", bufs=4)
                        for lc in range(NLC):
                            nc.tensor.matmul(
                                twp[:],
                                phT[:, lc * TOK + jg * 128:
                                    lc * TOK + jg * 128 + 128],
                                thT[:, lc * TOK + jg * 128:
                                    lc * TOK + jg * 128 + 128],
                                start=(lc == 0), stop=(lc == NLC - 1))
                        pend_tp.append((jg, twp))

                    def emit_tp():
                        jg, twp = pend_tp.pop(0)
                        sb = attnpool.tile([128, 128], F8E3, name="sb",
                                           tag="sb", bufs=3)
                        nc.vector.scalar_tensor_tensor(
                            out=sb[:], in0=twp[:], scalar=SB_SCALE,
                            in1=mask_sb[:], op0=OP.mult, op1=OP.mult)
                        pp = psmall.tile([128, 512], F32, name="ps_tp",
                                         tag="ps_small", bufs=4)
                        for lc in range(NLC):
                            nc.tensor.matmul(pp[:, ts(lc, 128)],
                                             g_act[jg][:, ts(lc, 128)], sb[:])
                        # pp free = (lc, j8, t16); stT t-major dst
                        # free = lc*2048 + t*128 + (8*jg + j)
                        dst = stT.rearrange("p (a t j) -> p a t j",
                                            a=NLC, t=T)[
                            :, :, :, ts(jg, JG)]
                        src = pp.rearrange("p (a j t) -> p a t j",
                                           a=NLC, j=JG)
                        nc.scalar.mul(dst, src, TP_SCALE)

                    def emit_gsp(i):
                        ps = pbig.tile([128, 512], F32, name="ps_g",
                                       tag="ps_big", bufs=4)
                        for cp in range(NCP):
                            nc.tensor.matmul(
                                ps[:],
                                xsp8[:, 2 * cp:2 * cp + 2, i:i + 1, :],
                                wgv[:, 2 * cp:2 * cp + 2, :],
                                start=(cp == 0), stop=(cp == NCP - 1),
                                perf_mode=DR)
                        nc.scalar.mul(g_sp[i][:], ps[:], G_DESCALE)

                    first_done = False
                    for k in range(NGRP):
                        emit_gsp(k)
                        emit_tw(k)
                        if not first_done and USE_COLLECTIVE:
                            # warm-up collective #2, keyed on 1c data so the
                            # CC stream is recently-used when the real stats
                            # collective triggers
                            nc.gpsimd.dma_start(cc_w2_in[:],
                                                g_sp[0][:, 0:1])
                            nc.gpsimd.collective_compute(
                                "AllGather", OP.bypass,
                                replica_groups=[list(range(N_CORES))],
                                ins=[cc_w2_in.opt()], outs=[cc_w2_out.opt()])
                            first_done = True
                        if len(pend_tp) >= 2:
                            emit_tp()
                    while pend_tp:
                        emit_tp()

                    # ------- phase 2: spatial attention (ADD into stT) ----
                    pend_sp = []   # (i, swp)

                    def sp_view(tile_ap, i):
                        return tile_ap.rearrange(
                            "p (j t) -> p t j", t=T)[:, i:i + 1, :]

                    def thsl(tt, lc):
                        return tt[:, lc * TOK:(lc + 1) * TOK]

                    def emit_sw(i):
                        swp = psmall.tile([128, 128], F32, name="ps_sw",
                                          tag="ps_small", bufs=4)
                        for lc in range(NLC):
                            nc.tensor.matmul(swp[:],
                                             sp_view(thsl(phT, lc), i),
                                             sp_view(thsl(thT, lc), i),
                                             start=(lc == 0),
                                             stop=(lc == NLC - 1))
                        pend_sp.append((i, swp))

                    def emit_sp():
                        i, swp = pend_sp.pop(0)
                        swb = attnpool.tile([128, 128], F8E3, name="swb",
                                            tag="swb", bufs=3)
                        nc.scalar.mul(swb[:], swp[:], SB_SCALE)
                        pp = psmall.tile([128, 512], F32, name="ps_sp",
                                         tag="ps_small", bufs=4)
                        for lc in range(NLC):
                            nc.tensor.matmul(pp[:, ts(lc, 128)],
                                             g_sp[i][:, ts(lc, 128)], swb[:])
                        # t-major stT: the t=i row (all 128 actors) is a
                        # contiguous 128-run per lc chunk -> fast DVE RMW
                        dst = stT.rearrange("p (a t j) -> p a t j",
                                            a=NLC, t=T)[:, :, i, :]
                        src = pp.rearrange("p (a j) -> p a j", a=NLC)
                        nc.vector.scalar_tensor_tensor(
                            out=dst, in0=src, scalar=SP_SCALE, in1=dst,
                            op0=OP.mult, op1=OP.add)

                    for i in range(T):
                        emit_sw(i)
                        if len(pend_sp) >= 2:
                            emit_sp()
                    while pend_sp:
                        emit_sp()

            # ------- phase 3: out-projection + residual + stats -------
            with tc.tile_pool(name="outp", bufs=1) as outpool, \
                 tc.tile_pool(name="yp", bufs=1) as ypool, \
                 tc.tile_pool(name="sqp", bufs=1) as sqpool:
                out_sb = []
                inv_n = 1.0 / float(NTOK_GLOBAL)
                # t-major stT viewed as (lc, t, j); the moving operand for
                # token chunk tck walks (jj outer stride 1, t inner
                # stride 128) so PSUM free order stays token-major (j,t)
                stv = stT.rearrange("p (a t j) -> p a j t", a=NLC, t=T)

                def emit_outproj(ct):
                    o = outpool.tile([128, TOK], BF16, name=f"out{ct}",
                                     tag=f"out{ct}")
                    out_sb.append(o)
                    # tck-inner with 4 concurrent PSUM groups: each ww
                    # weight tile serves 4 back-to-back matmuls
                    pss = [pbig.tile([128, 512], F32, name=f"ps_out{t}",
                                     tag="ps_big", bufs=4)
                           for t in range(NTC)]
                    for lc in range(NLC):
                        for tck in range(NTC):
                            nc.tensor.matmul(
                                pss[tck][:],
                                ww_all[:, lc * C + ct * 128:
                                       lc * C + (ct + 1) * 128],
                                stv[:, lc, ts(tck, 32), :],
                                start=(lc == 0), stop=(lc == NLC - 1))
                    for tck in range(NTC):
                        col = ct * NTC + tck
                        nc.vector.scalar_tensor_tensor(
                            out=o[:, ts(tck, 512)], in0=pss[tck][:],
                            scalar=OUT_DESCALE,
                            in1=xbf.rearrange("p (tk a k) -> p tk a k",
                                              tk=NTC, a=NCC)[:, tck, ct, :],
                            op0=OP.mult, op1=OP.add,
                            accum_out=stat_sum[:, col:col + 1])
                        sq = sqpool.tile([128, 512], F32, name="sqscr",
                                         tag="sq", bufs=3)
                        nc.scalar.activation(
                            sq[:], o[:, ts(tck, 512)], ACT_FN.Square,
                            accum_out=stat_sq[:, col:col + 1])

                def emit_stats_cc():
                    """AllGather per-channel sum/sumsq + local reduce."""
                    nc.vector.tensor_reduce(
                        red_in[:, 0:8],
                        stat_sum.rearrange("p (a b) -> p a b", a=8)[:, :, :],
                        axis=AX.X, op=OP.add)
                    nc.vector.tensor_reduce(
                        red_in[:, 8:16],
                        stat_sq.rearrange("p (a b) -> p a b", a=8)[:, :, :],
                        axis=AX.X, op=OP.add)
                    if USE_COLLECTIVE:
                        nc.gpsimd.dma_start(cc_big_in[:], red_in[:])
                        nc.gpsimd.collective_compute(
                            "AllGather", OP.bypass,
                            replica_groups=[list(range(N_CORES))],
                            ins=[cc_big_in.opt()], outs=[cc_big_out.opt()])
                        # cc_big_out is [8*128, 16]: rank r at rows
                        # [r*128, (r+1)*128) -> SBUF [p, (r, col)]
                        src = cc_big_out.rearrange("(r p) c -> p r c",
                                                   r=N_CORES)
                        dst = ag_sb.rearrange("p (r c) -> p r c",
                                              r=N_CORES)
                        nc.gpsimd.dma_start(dst, src)
                        nc.vector.tensor_reduce(
                            red_out[:, 0:16],
                            ag_sb.rearrange("p (r c) -> p c r",
                                            r=N_CORES)[:, :, :],
                            axis=AX.X, op=OP.add)
                    else:
                        nc.vector.tensor_scalar_mul(
                            red_out[:], red_in[:], float(N_CORES))

                def emit_bn_params():
                    lo, hi, part = 0, 8, 0
                    mean = statpool.tile([128, 8], F32, name=f"mean{part}",
                                         tag=f"mean{part}")
                    var = statpool.tile([128, 8], F32, name=f"var{part}",
                                        tag=f"var{part}")
                    std = statpool.tile([128, 8], F32, name=f"std{part}",
                                        tag=f"std{part}")
                    rstd = statpool.tile([128, 8], F32, name=f"rstd{part}",
                                         tag=f"rstd{part}")
                    nc.vector.tensor_scalar_mul(mean[:], red_out[:, lo:hi],
                                                inv_n)
                    nc.vector.tensor_scalar_mul(var[:],
                                                red_out[:, 8 + lo:8 + hi],
                                                inv_n)
                    nc.vector.tensor_mul(std[:], mean[:], mean[:])
                    nc.vector.tensor_tensor(var[:], var[:], std[:],
                                            op=OP.subtract)
                    nc.vector.tensor_scalar_add(var[:], var[:], BN_EPS)
                    nc.scalar.activation(std[:], var[:], ACT_FN.Sqrt, bias=0.0)
                    nc.vector.reciprocal(rstd[:], std[:])
                    nc.vector.tensor_mul(scalev[:, lo:hi], rstd[:],
                                         gb_sb[:, lo:hi])
                    nc.vector.tensor_mul(rstd[:], mean[:], scalev[:, lo:hi])
                    nc.vector.tensor_tensor(biasv[:, lo:hi],
                                            gb_sb[:, 8 + lo:8 + hi], rstd[:],
                                            op=OP.subtract)

                def emit_apply(ct):
                    # DVE is ~2x faster per op here than ACT: give DVE 11 of
                    # 16 half-tiles, ACT 5; stores spread over the sync,
                    # scalar and gpsimd HWDGE rings (ring speeds vary run to
                    # run); bf16 stores (host upcasts)
                    for h in range(2):
                        k = (2 * ct + h) % 3
                        src = out_sb[ct][:, ts(h, 1024)]
                        if k == 2:
                            y = ypool.tile([128, 1024], BF16, name="ya",
                                           tag="ya", bufs=6)
                            nc.scalar.activation(
                                y[:], src, ACT_FN.Identity,
                                scale=scalev[:, ct:ct + 1],
                                bias=biasv[:, ct:ct + 1])
                            nc.scalar.dma_start(
                                outy_d[ts(ct, 128), ts(h, 1024)], y[:])
                        else:
                            y = ypool.tile([128, 1024], BF16, name="yb",
                                           tag="yb", bufs=6)
                            nc.vector.tensor_scalar(
                                out=y[:], in0=src,
                                scalar1=scalev[:, ct:ct + 1],
                                scalar2=biasv[:, ct:ct + 1],
                                op0=OP.mult, op1=OP.add)
                            if k == 0:
                                nc.sync.dma_start(
                                    outy_d[ts(ct, 128), ts(h, 1024)], y[:])
                            else:
                                nc.gpsimd.dma_start(
                                    outy_d[ts(ct, 128), ts(h, 1024)], y[:])

                for ct in range(NCC):
                    emit_outproj(ct)
                emit_stats_cc()
                emit_bn_params()
                for ct in range(NCC):
                    emit_apply(ct)

    nc.compile()
    return nc


def _get_compiled():
    global _compiled
    if _compiled is None:
        _compiled = _build()
    return _compiled


def kernel(x, Wt, Wp, Wg, Ww, gamma, beta, _trace=False, _trace_kwargs=None):
    global _last_results
    nc = _get_compiled()

    x = np.asarray(x, dtype=np.float32)
    Wt = np.asarray(Wt, dtype=np.float32)
    Wp = np.asarray(Wp, dtype=np.float32)
    Wg = np.asarray(Wg, dtype=np.float32)
    Ww = np.asarray(Ww, dtype=np.float32)
    gamma = np.asarray(gamma, dtype=np.float32)
    beta = np.asarray(beta, dtype=np.float32)

    bf = ml_dtypes.bfloat16
    f8e4 = ml_dtypes.float8_e4m3
    f8e3 = ml_dtypes.float8_e3m4

    def shuf(a):
        """[n*128, F] -> tile layout [128, n*F] (chunk-major free axis)."""
        n = a.shape[0] // 128
        return np.ascontiguousarray(
            a.reshape(n, 128, a.shape[1]).transpose(1, 0, 2).reshape(128, -1))

    def shuf_x(a):
        """[C, TOK] -> token-chunk-major tile layout [128, (tck, a, 512)]."""
        return np.ascontiguousarray(
            a.reshape(NCC, 128, NTC, 512).transpose(1, 2, 0, 3)
            .reshape(128, -1))

    wt_t = shuf((Wt.T * WG_SCALE).astype(f8e4))       # [C, L] -> tile
    wp_t = shuf(Wp.T.astype(bf))
    wg_t = shuf((Wg.T * WG_SCALE).astype(f8e4))
    ww_t = shuf((Ww.T * WW_SCALE).astype(f8e3))       # [L, C] -> tile
    r = np.arange(128)
    mask = (r[:, None] // T == r[None, :] // T).astype(bf)
    gb = np.concatenate(
        [gamma.reshape(NCC, 128).T,
         beta.reshape(NCC, 128).T], axis=1).astype(np.float32)  # [128, 16]

    # actor-major token order: tok = j*T + t
    xa = x.transpose(0, 2, 1, 3).reshape(B, TOK, C)
    in_maps = []
    for b in range(B):
        xT = np.ascontiguousarray(xa[b].T)            # [C, TOK] f32
        x8 = xT.astype(f8e4)
        in_maps.append(dict(
            xf8=shuf_x(x8), xf8b=shuf(x8), xbf=shuf_x(xT.astype(bf)),
            wt=wt_t, wp=wp_t, wg=wg_t, ww=ww_t,
            mask=mask, gb=gb))

    res = run_bass_kernel_spmd(nc, in_maps, list(range(N_CORES)),
                               trace=_trace, **(_trace_kwargs or {}))
    _last_results = res

    ys = []
    for b in range(B):
        o = np.asarray(res.results[b]["outy"], dtype=np.float32)   # [C, TOK]
        ys.append(o.T.reshape(N, T, C).transpose(1, 0, 2))          # [T, N, C]
    return np.stack(ys)


# revision 13
# speedup vs baseline: 1.2535x; 1.2535x over previous
"""Trainium2 Bass kernel for CrossInferBlock (spatial+temporal cross attention
+ out-projection + residual + BatchNorm over (B,T,N)).

Sharding: data-parallel over B across 8 NeuronCores (one batch element per
core). BN batch statistics are combined via an 8-core AllGather (8KB ->
64KB) + a local DVE reduce -- measured ~4x faster than AllReduce on this
fabric (AllReduce has a ~32us fixed cost; AllGather ~a third of that).

Precision plan (the residual dominates the output; the attention branch is
~17% of output magnitude, so fp8 there is cheap in accuracy; measured
rel err 1.6e-2 vs the fp32 reference, gate 2e-2):
  - theta + g projections: fp8e4 DoubleRow (2 K-tiles/pass = 2x PE
    throughput); x/Wt/Wg uploaded e4m3 (weights x16), outputs stored e3m4.
  - phi projection: bf16 (phi/theta errors multiply in the attention
    scores, so one of the pair stays high-precision).
  - attention scores tw/sw -> e3m4 (x1/32); applies tp/sp run fp8 at the
    bf16 rate; stT stored e3m4 (x8, max |stT| = 11.5 < 15.5).
  - out-projection: e3m4 x e3m4 (Ww x32); PSUM descaled by 2^-8 at the
    bf16 residual add. BN stats/apply in fp32.
All scale factors are powers of two (exact).

Device-side token order is ACTOR-MAJOR: tok = j*T + t. stT however is
stored T-MAJOR (free = lc*TOK + t*128 + j): the 16 per-timestep spatial
read-modify-write adds (phase 2, on the congested DVE) become contiguous
(~0.7us vs 2.6us strided), while the 16 temporal init writes eat the
stride on the half-idle ACT engine in phase 1. The out-projection reads
stT through a strided moving AP (stride-128 over t), which the PE walks
at full rate, so PSUM/residual/output stay token-major.

Phase order is chosen for DMA just-in-time: g_act+theta (needs only
wg/wt/xf8 = 3MB) start ~13us in while phi's inputs (wp/xbf = 5MB) and
the phase-2/3 tensors (xf8b, ww) stream in behind. theta and phi run
lc-major against 4 (2 for phi) concurrent PSUM accumulation groups so
each stationary weight tile is loaded once, not once per token chunk
(saves ~20k LDWEIGHTS columns). g_sp is precomputed in phase 1c so
phase 2 is only sw/sp + the cheap contiguous RMW.

Collectives: one warm-up AllGather at load time absorbs the CC stream's
one-time ~26us setup; a second keyed on phase-1c data keeps the ring
recent. The real stats AllGather fires as soon as the last out-projection
epilogue lands. BN apply+store is split across the DVE and ACT engines
with bf16 stores on three HWDGE rings (the host upcasts to fp32).
"""

import sys

if "/opt/trn_rl_repo" not in sys.path:
    sys.path.insert(0, "/opt/trn_rl_repo")

import numpy as np
import ml_dtypes

import concourse.bass as bass
import concourse.bacc as bacc
import concourse.tile as tile
import concourse.mybir as mybir
from concourse.bass_utils import run_bass_kernel_spmd
from contextlib import ExitStack

F32 = mybir.dt.float32
BF16 = mybir.dt.bfloat16
F8E4 = mybir.dt.float8e4     # e4m3: DoubleRow-capable
F8E3 = mybir.dt.float8e3     # e3m4: 2x mantissa, bf16-rate matmuls
AX = mybir.AxisListType
OP = mybir.AluOpType
ACT_FN = mybir.ActivationFunctionType
DR = mybir.MatmulPerfMode.DoubleRow

N_CORES = 8
B, T, N, C = 8, 16, 128, 1024
L = C // 2            # 512
TOK = T * N           # 2048 tokens per batch element
NTOK_GLOBAL = B * T * N
JG = 8                # actors per temporal group
NGRP = N // JG        # 16 groups
BN_EPS = 1e-5

WG_SCALE = 16.0       # Wg uploaded x16 (e4m3)
WW_SCALE = 32.0       # Ww uploaded x32 (e3m4: normal range starts at 0.25)
G_DESCALE = 1.0 / WG_SCALE
SB_SCALE = 1.0 / 32.0            # attention scores into e3m4 (std ~1.4)
STT_SCALE = 8.0                  # stT e3m4 boost (max|stT|=11.5 < 15.5)
SP_SCALE = STT_SCALE / (N * (T + N)) / SB_SCALE    # 0.027778
TP_SCALE = STT_SCALE / (T * (T + N)) / SB_SCALE    # 0.222
OUT_DESCALE = 1.0 / (STT_SCALE * WW_SCALE)         # 2^-9

NCC = C // 128     # 8 c-chunks
NLC = L // 128     # 4 l-chunks
NCP = NCC // 2     # 4 c-chunk pairs (DoubleRow)
NTC = TOK // 512   # 4 token chunks

_compiled = None
_last_results = None

USE_COLLECTIVE = True


def ts(i, size):
    return bass.ts(i, size)


def _build():
    nc = bacc.Bacc("TRN2", target_bir_lowering=False, debug=False,
                   num_devices=N_CORES)

    # ---- DRAM I/O (token order: actor-major, tok = j*T + t) ----
    # inputs are pre-shuffled on the host into the SBUF tile layout
    # [128, chunk*free] so every load is a full-row (4-32KB/row) DMA
    xf8_d = nc.dram_tensor("xf8", [128, NCC * TOK], F8E4,
                           kind="ExternalInput")
    # second fp8 x copy in channel-major layout: the spatial projection's
    # stride-T token gather needs (a, tok) order, which would make the
    # token-chunk-major tile a 4-free-dim DoubleRow weights AP
    xf8b_d = nc.dram_tensor("xf8b", [128, NCC * TOK], F8E4,
                            kind="ExternalInput")
    xbf_d = nc.dram_tensor("xbf", [128, NCC * TOK], BF16,
                           kind="ExternalInput")
    wt_d = nc.dram_tensor("wt", [128, NCC * L], F8E4, kind="ExternalInput")
    wp_d = nc.dram_tensor("wp", [128, NCC * L], BF16, kind="ExternalInput")
    wg_d = nc.dram_tensor("wg", [128, NCC * L], F8E4, kind="ExternalInput")
    ww_d = nc.dram_tensor("ww", [128, NLC * C], F8E3, kind="ExternalInput")
    mask_d = nc.dram_tensor("mask", [128, 128], BF16, kind="ExternalInput")
    gb_d = nc.dram_tensor("gb", [128, 16], F32, kind="ExternalInput")
    outy_d = nc.dram_tensor("outy", [C, TOK], BF16, kind="ExternalOutput")

    with tile.TileContext(nc) as tc:
        with ExitStack() as outer:
            # ---------------- persistent pools ----------------
            cpool = outer.enter_context(tc.tile_pool(name="consts", bufs=1))
            wwpool = outer.enter_context(tc.tile_pool(name="wwp", bufs=1))
            stpool = outer.enter_context(tc.tile_pool(name="stp", bufs=1))
            statpool = outer.enter_context(tc.tile_pool(name="stats", bufs=1))
            pbig = outer.enter_context(
                tc.tile_pool(name="pbig", bufs=1, space="PSUM"))
            psmall = outer.enter_context(
                tc.tile_pool(name="psmall", bufs=1, space="PSUM"))
            drampool = outer.enter_context(
                tc.tile_pool(name="dramp", bufs=1, space="DRAM"))
            xbpool = outer.enter_context(tc.tile_pool(name="xbp", bufs=1))

            mask_sb = cpool.tile([128, 128], BF16, name="mask_sb",
                                 tag="mask_sb")
            gb_sb = cpool.tile([128, 16], F32, name="gb_sb", tag="gb_sb")
            ww_all = wwpool.tile([128, NLC * C], F8E3, name="ww_all", tag="ww")
            # stT is T-MAJOR: free = lc*TOK + t*128 + j
            stT = stpool.tile([128, NLC * TOK], F8E3, name="stT", tag="stT")

            stat_sum = statpool.tile([128, 32], F32, name="stat_sum",
                                     tag="stat_sum")
            stat_sq = statpool.tile([128, 32], F32, name="stat_sq",
                                    tag="stat_sq")
            red_in = statpool.tile([128, 16], F32, name="red_in", tag="red_in")
            ag_sb = statpool.tile([128, N_CORES * 16], F32, name="ag_sb",
                                  tag="ag_sb")
            red_out = statpool.tile([128, 16], F32, name="red_out",
                                    tag="red_out")
            scalev = statpool.tile([128, 8], F32, name="scalev", tag="scalev")
            biasv = statpool.tile([128, 8], F32, name="biasv", tag="biasv")

            cc_big_in = drampool.tile([128, 16], F32, name="cc_big_in",
                                      tag="cc_big_in")
            cc_big_out = drampool.tile([N_CORES * 128, 16], F32,
                                       name="cc_big_out", tag="cc_big_out")
            cc_warm_in = drampool.tile([128, 1], F32, name="cc_warm_in",
                                       tag="cc_warm_in")
            cc_warm_out = drampool.tile([N_CORES * 128, 1], F32,
                                        name="cc_warm_out", tag="cc_warm_out")
            cc_w2_in = drampool.tile([128, 1], F32, name="cc_w2_in",
                                     tag="cc_w2_in")
            cc_w2_out = drampool.tile([N_CORES * 128, 1], F32,
                                      name="cc_w2_out", tag="cc_w2_out")

            with ExitStack() as mid:
                thpool = mid.enter_context(tc.tile_pool(name="thp", bufs=1))
                gpool = mid.enter_context(tc.tile_pool(name="gp", bufs=1))
                attnpool = mid.enter_context(tc.tile_pool(name="attn", bufs=1))

                thT = thpool.tile([128, NLC * TOK], F8E3, name="thT",
                                  tag="thT")
                phT = thpool.tile([128, NLC * TOK], F8E3, name="phT",
                                  tag="phT")
                g_sp = [gpool.tile([128, L], F8E3, name=f"gsp{i}",
                                   tag=f"gsp{i}") for i in range(T)]
                g_act = [gpool.tile([128, L], F8E3, name=f"gact{j}",
                                    tag=f"gact{j}") for j in range(NGRP)]

                with ExitStack() as phase_a:
                    wpool = phase_a.enter_context(
                        tc.tile_pool(name="wp", bufs=1))

                    xf8 = xbpool.tile([128, NCC * TOK], F8E4, name="xf8",
                                      tag="xf8")
                    xf8b = xbpool.tile([128, NCC * TOK], F8E4, name="xf8b",
                                       tag="xf8b")
                    xbf = xbpool.tile([128, NCC * TOK], BF16, name="xbf",
                                      tag="xbf")
                    wt_all = wpool.tile([128, NCC * L], F8E4, name="wt_all",
                                        tag="wt")
                    wp_all = wpool.tile([128, NCC * L], BF16, name="wp_all",
                                        tag="wp")
                    wg_all = wpool.tile([128, NCC * L], F8E4, name="wg_all",
                                        tag="wg")

                    # input DMA schedule, critical-path first on ALL three
                    # rings (each ring gets ~1/3 of HBM bandwidth when all
                    # are busy): the 3MB gating phase 1a (wg/wt/xf8) is
                    # spread across the rings in front position; phi's
                    # inputs (wp/xbf) follow; xf8b (phase 1c) and ww
                    # (phase 3) ride last.
                    CHW = NCC * 512          # flat cols per token chunk
                    nc.gpsimd.dma_start(wg_all[:], wg_d[:])
                    nc.scalar.dma_start(xf8[:, ts(1, CHW)],
                                        xf8_d[:, ts(1, CHW)])
                    nc.sync.dma_start(xf8[:, 0:CHW], xf8_d[:, 0:CHW])
                    nc.gpsimd.dma_start(xf8[:, ts(2, CHW)],
                                        xf8_d[:, ts(2, CHW)])
                    nc.scalar.dma_start(wt_all[:], wt_d[:])
                    nc.gpsimd.dma_start(xf8[:, ts(3, CHW)],
                                        xf8_d[:, ts(3, CHW)])
                    nc.sync.dma_start(wp_all[:], wp_d[:])
                    nc.gpsimd.dma_start(mask_sb[:], mask_d[:])
                    nc.gpsimd.dma_start(gb_sb[:], gb_d[:])
                    for tk in range(2):
                        nc.scalar.dma_start(xbf[:, ts(tk, CHW)],
                                            xbf_d[:, ts(tk, CHW)])
                    for tk in range(2, NTC):
                        nc.sync.dma_start(xbf[:, ts(tk, CHW)],
                                          xbf_d[:, ts(tk, CHW)])
                    nc.scalar.dma_start(xf8b[:], xf8b_d[:])
                    nc.sync.dma_start(ww_all[:], ww_d[:])
                    if USE_COLLECTIVE:
                        # warm-up collective #1: pays the CC stream's
                        # one-time setup during the DMA load
                        nc.gpsimd.dma_start(cc_warm_in[:], gb_d[:, 0:1])
                        nc.gpsimd.collective_compute(
                            "AllGather", OP.bypass,
                            replica_groups=[list(range(N_CORES))],
                            ins=[cc_warm_in.opt()], outs=[cc_warm_out.opt()])

                    # views (x tiles are token-chunk-major: (tck, a, k))
                    xv8 = xf8.rearrange("p (tk a k) -> p tk a k",
                                        tk=NTC, a=NCC)
                    # spatial: tok = j*T + t (channel-major copy)
                    xsp8 = xf8b.rearrange("p (a j t) -> p a t j",
                                          a=NCC, t=T)
                    xbv = xbf.rearrange("p (tk a k) -> p tk a k",
                                        tk=NTC, a=NCC)
                    wgv = wg_all.rearrange("p (a l) -> p a l", a=NCC)
                    wtv = wt_all.rearrange("p (a l) -> p a l", a=NCC)

                    def xsl(c, tck):
                        return xbv[:, tck, c, :]

                    def wsl(w, c, lc):
                        return w[:, c * L + lc * 128:c * L + (lc + 1) * 128]

                    # ------- phase 1a: g_act (xf8+wg) then theta (wt) ------
                    for jg in range(NGRP):
                        tck = jg // 4
                        ps = pbig.tile([128, 512], F32, name="ps_ga",
                                       tag="ps_big", bufs=4)
                        for cp in range(NCP):
                            nc.tensor.matmul(
                                ps[:],
                                xv8[:, tck, 2 * cp:2 * cp + 2,
                                    ts(jg - 4 * tck, 128)],
                                wgv[:, 2 * cp:2 * cp + 2, :],
                                start=(cp == 0), stop=(cp == NCP - 1),
                                perf_mode=DR)
                        nc.scalar.mul(g_act[jg][:], ps[:], G_DESCALE)

                    # theta: fp8 DoubleRow, lc-major with 4 concurrent tck
                    # PSUM groups -- each wt tile is loaded once, serving
                    # 4 back-to-back matmuls (consecutive loads dedupe)
                    for lc in range(NLC):
                        pss = [pbig.tile([128, 512], F32, name=f"ps_th{t}",
                                         tag="ps_big", bufs=4)
                               for t in range(NTC)]
                        for cp in range(NCP):
                            for tck in range(NTC):
                                nc.tensor.matmul(
                                    pss[tck][:],
                                    wtv[:, 2 * cp:2 * cp + 2, ts(lc, 128)],
                                    xv8[:, tck, 2 * cp:2 * cp + 2, :],
                                    start=(cp == 0), stop=(cp == NCP - 1),
                                    perf_mode=DR)
                        for tck in range(NTC):
                            dst = thT[:, lc * TOK + tck * 512:
                                      lc * TOK + tck * 512 + 512]
                            nc.vector.tensor_scalar_mul(dst, pss[tck][:],
                                                        G_DESCALE)

                    # ------- phase 1b: phi (bf16; wp + xbf) ---------------
                    # xbf is T-MAJOR (tq = t//4 quarter chunks) so the
                    # out-projection's residual add matches the contiguous
                    # t-major stT reads; phi's PSUM therefore comes out
                    # t-major and is scattered into the actor-major phT
                    # (stride-16 writes, alternating DVE/ACT -- both
                    # half-idle here, hidden under phi's 36us of matmul).
                    # lc-major over tq pairs (2 concurrent PSUM groups) so
                    # phi can start once xbf chunks 0-1 have landed.
                    phTv = phT.rearrange("p (a j t) -> p a t j",
                                         a=NLC, t=T)
                    for half in range(2):
                        tqs = (2 * half, 2 * half + 1)
                        for lc in range(NLC):
                            pss = [pbig.tile([128, 512], F32,
                                             name=f"ps_ph{t}",
                                             tag="ps_big", bufs=4)
                                   for t in tqs]
                            for c in range(NCC):
                                for k, tq in enumerate(tqs):
                                    nc.tensor.matmul(
                                        pss[k][:], wsl(wp_all, c, lc),
                                        xsl(c, tq),
                                        start=(c == 0), stop=(c == NCC - 1))
                            for k, tq in enumerate(tqs):
                                dst = phTv[:, lc, 4 * tq:4 * tq + 4, :]
                                src = pss[k].rearrange("p (t j) -> p t j",
                                                       t=4)
                                if lc % 2 == 0:
                                    nc.vector.tensor_copy(dst, src)
                                else:
                                    nc.scalar.copy(dst, src)

                    # ---- phase 1c: temporal attention + g_sp precompute --
                    # temporal INITIALIZES stT (t-major, strided write on the
                    # half-idle ACT engine); g_sp precomputed here so phase 2
                    # is only sw/sp
                    pend_tp = []   # (jg, twp)

                    def emit_tw(jg):
                        twp = psmall.tile([128, 128], F32, name="ps_tw",
                                          tag="ps_small", bufs=4)
                        for lc in range(NLC):
                            nc.tensor.matmul(
                                twp[:],
                                phT[:, lc * TOK + jg * 128:
                                    lc * TOK + jg * 128 + 128],
                                thT[:, lc * TOK + jg * 128:
                                    lc * TOK + jg * 128 + 128],
                                start=(lc == 0), stop=(lc == NLC - 1))
                        pend_tp.append((jg, twp))

                    def emit_tp():
                        jg, twp = pend_tp.pop(0)
                        sb = attnpool.tile([128, 128], F8E3, name="sb",
                                           tag="sb", bufs=3)
                        nc.vector.scalar_tensor_tensor(
                            out=sb[:], in0=twp[:], scalar=SB_SCALE,
                            in1=mask_sb[:], op0=OP.mult, op1=OP.mult)
                        pp = psmall.tile([128, 512], F32, name="ps_tp",
                                         tag="ps_small", bufs=4)
                        for lc in range(NLC):
                            nc.tensor.matmul(pp[:, ts(lc, 128)],
                                             g_act[jg][:, ts(lc, 128)], sb[:])
                        # pp free = (lc, j8, t16); stT t-major dst
                        # free = lc*2048 + t*128 + (8*jg + j)
                        dst = stT.rearrange("p (a t j) -> p a t j",
                                            a=NLC, t=T)[
                            :, :, :, ts(jg, JG)]
                        src = pp.rearrange("p (a j t) -> p a t j",
                                           a=NLC, j=JG)
                        nc.scalar.mul(dst, src, TP_SCALE)

                    def emit_gsp(i):
                        ps = pbig.tile([128, 512], F32, name="ps_g",
                                       tag="ps_big", bufs=4)
                        for cp in range(NCP):
                            nc.tensor.matmul(
                                ps[:],
                                xsp8[:, 2 * cp:2 * cp + 2, i:i + 1, :],
                                wgv[:, 2 * cp:2 * cp + 2, :],
                                start=(cp == 0), stop=(cp == NCP - 1),
                                perf_mode=DR)
                        nc.scalar.mul(g_sp[i][:], ps[:], G_DESCALE)

                    first_done = False
                    for k in range(NGRP):
                        emit_gsp(k)
                        emit_tw(k)
                        if not first_done and USE_COLLECTIVE:
                            # warm-up collective #2, keyed on 1c data so the
                            # CC stream is recently-used when the real stats
                            # collective triggers
                            nc.gpsimd.dma_start(cc_w2_in[:],
                                                g_sp[0][:, 0:1])
                            nc.gpsimd.collective_compute(
                                "AllGather", OP.bypass,
                                replica_groups=[list(range(N_CORES))],
                                ins=[cc_w2_in.opt()], outs=[cc_w2_out.opt()])
                            first_done = True
                        if len(pend_tp) >= 2:
                            emit_tp()
                    while pend_tp:
                        emit_tp()

                    # ------- phase 2: spatial attention (ADD into stT) ----
                    pend_sp = []   # (i, swp)

                    def sp_view(tile_ap, i):
                        return tile_ap.rearrange(
                            "p (j t) -> p t j", t=T)[:, i:i + 1, :]

                    def thsl(tt, lc):
                        return tt[:, lc * TOK:(lc + 1) * TOK]

                    def emit_sw(i):
                        swp = psmall.tile([128, 128], F32, name="ps_sw",
                                          tag="ps_small", bufs=4)
                        for lc in range(NLC):
                            nc.tensor.matmul(swp[:],
                                             sp_view(thsl(phT, lc), i),
                                             sp_view(thsl(thT, lc), i),
                                             start=(lc == 0),
                                             stop=(lc == NLC - 1))
                        pend_sp.append((i, swp))

                    def emit_sp():
                        i, swp = pend_sp.pop(0)
                        swb = attnpool.tile([128, 128], F8E3, name="swb",
                                            tag="swb", bufs=3)
                        nc.scalar.mul(swb[:], swp[:], SB_SCALE)
                        pp = psmall.tile([128, 512], F32, name="ps_sp",
                                         tag="ps_small", bufs=4)
                        for lc in range(NLC):
                            nc.tensor.matmul(pp[:, ts(lc, 128)],
                                             g_sp[i][:, ts(lc, 128)], swb[:])
                        # t-major stT: the t=i row (all 128 actors) is a
                        # contiguous 128-run per lc chunk -> fast DVE RMW
                        dst = stT.rearrange("p (a t j) -> p a t j",
                                            a=NLC, t=T)[:, :, i, :]
                        src = pp.rearrange("p (a j) -> p a j", a=NLC)
                        nc.vector.scalar_tensor_tensor(
                            out=dst, in0=src, scalar=SP_SCALE, in1=dst,
                            op0=OP.mult, op1=OP.add)

                    for i in range(T):
                        emit_sw(i)
                        if len(pend_sp) >= 2:
                            emit_sp()
                    while pend_sp:
                        emit_sp()

            # ------- phase 3: out-projection + residual + stats -------
            with tc.tile_pool(name="outp", bufs=1) as outpool, \
                 tc.tile_pool(name="yp", bufs=1) as ypool, \
                 tc.tile_pool(name="sqp", bufs=1) as sqpool:
                out_sb = []
                inv_n = 1.0 / float(NTOK_GLOBAL)

                def emit_outproj(ct):
                    o = outpool.tile([128, TOK], BF16, name=f"out{ct}",
                                     tag=f"out{ct}")
                    out_sb.append(o)
                    # tq-inner with 4 concurrent PSUM groups: each ww
                    # weight tile serves 4 back-to-back matmuls; the moving
                    # operand is a CONTIGUOUS 512-col t-major stT slice
                    # (tokens t in [4tq, 4tq+4), all actors), matching the
                    # t-major xbf/output layout
                    pss = [pbig.tile([128, 512], F32, name=f"ps_out{t}",
                                     tag="ps_big", bufs=4)
                           for t in range(NTC)]
                    for lc in range(NLC):
                        for tq in range(NTC):
                            nc.tensor.matmul(
                                pss[tq][:],
                                ww_all[:, lc * C + ct * 128:
                                       lc * C + (ct + 1) * 128],
                                stT[:, lc * TOK + tq * 512:
                                    lc * TOK + tq * 512 + 512],
                                start=(lc == 0), stop=(lc == NLC - 1))
                    for tq in range(NTC):
                        col = ct * NTC + tq
                        nc.vector.scalar_tensor_tensor(
                            out=o[:, ts(tq, 512)], in0=pss[tq][:],
                            scalar=OUT_DESCALE,
                            in1=xbf.rearrange("p (tk a k) -> p tk a k",
                                              tk=NTC, a=NCC)[:, tq, ct, :],
                            op0=OP.mult, op1=OP.add,
                            accum_out=stat_sum[:, col:col + 1])
                        sq = sqpool.tile([128, 512], F32, name="sqscr",
                                         tag="sq", bufs=3)
                        nc.scalar.activation(
                            sq[:], o[:, ts(tq, 512)], ACT_FN.Square,
                            accum_out=stat_sq[:, col:col + 1])

                def emit_stats_cc():
                    """AllGather per-channel sum/sumsq + local reduce."""
                    nc.vector.tensor_reduce(
                        red_in[:, 0:8],
                        stat_sum.rearrange("p (a b) -> p a b", a=8)[:, :, :],
                        axis=AX.X, op=OP.add)
                    nc.vector.tensor_reduce(
                        red_in[:, 8:16],
                        stat_sq.rearrange("p (a b) -> p a b", a=8)[:, :, :],
                        axis=AX.X, op=OP.add)
                    if USE_COLLECTIVE:
                        nc.gpsimd.dma_start(cc_big_in[:], red_in[:])
                        nc.gpsimd.collective_compute(
                            "AllGather", OP.bypass,
                            replica_groups=[list(range(N_CORES))],
                            ins=[cc_big_in.opt()], outs=[cc_big_out.opt()])
                        # cc_big_out is [8*128, 16]: rank r at rows
                        # [r*128, (r+1)*128) -> SBUF [p, (r, col)]
                        src = cc_big_out.rearrange("(r p) c -> p r c",
                                                   r=N_CORES)
                        dst = ag_sb.rearrange("p (r c) -> p r c",
                                              r=N_CORES)
                        nc.gpsimd.dma_start(dst, src)
                        nc.vector.tensor_reduce(
                            red_out[:, 0:16],
                            ag_sb.rearrange("p (r c) -> p c r",
                                            r=N_CORES)[:, :, :],
                            axis=AX.X, op=OP.add)
                    else:
                        nc.vector.tensor_scalar_mul(
                            red_out[:], red_in[:], float(N_CORES))

                def emit_bn_params():
                    lo, hi, part = 0, 8, 0
                    mean = statpool.tile([128, 8], F32, name=f"mean{part}",
                                         tag=f"mean{part}")
                    var = statpool.tile([128, 8], F32, name=f"var{part}",
                                        tag=f"var{part}")
                    std = statpool.tile([128, 8], F32, name=f"std{part}",
                                        tag=f"std{part}")
                    rstd = statpool.tile([128, 8], F32, name=f"rstd{part}",
                                         tag=f"rstd{part}")
                    nc.vector.tensor_scalar_mul(mean[:], red_out[:, lo:hi],
                                                inv_n)
                    nc.vector.tensor_scalar_mul(var[:],
                                                red_out[:, 8 + lo:8 + hi],
                                                inv_n)
                    nc.vector.tensor_mul(std[:], mean[:], mean[:])
                    nc.vector.tensor_tensor(var[:], var[:], std[:],
                                            op=OP.subtract)
                    nc.vector.tensor_scalar_add(var[:], var[:], BN_EPS)
                    nc.scalar.activation(std[:], var[:], ACT_FN.Sqrt, bias=0.0)
                    nc.vector.reciprocal(rstd[:], std[:])
                    nc.vector.tensor_mul(scalev[:, lo:hi], rstd[:],
                                         gb_sb[:, lo:hi])
                    nc.vector.tensor_mul(rstd[:], mean[:], scalev[:, lo:hi])
                    nc.vector.tensor_tensor(biasv[:, lo:hi],
                                            gb_sb[:, 8 + lo:8 + hi], rstd[:],
                                            op=OP.subtract)

                def emit_apply(ct):
                    # DVE is ~2.3x faster per op here than ACT: give DVE 12
                    # of 16 half-tiles, ACT 4; stores spread over the sync,
                    # scalar and gpsimd HWDGE rings (ring speeds vary run to
                    # run); bf16 stores (host upcasts)
                    for h in range(2):
                        k = (2 * ct + h) % 4
                        src = out_sb[ct][:, ts(h, 1024)]
                        if k == 3:
                            y = ypool.tile([128, 1024], BF16, name="ya",
                                           tag="ya", bufs=6)
                            nc.scalar.activation(
                                y[:], src, ACT_FN.Identity,
                                scale=scalev[:, ct:ct + 1],
                                bias=biasv[:, ct:ct + 1])
                            nc.scalar.dma_start(
                                outy_d[ts(ct, 128), ts(h, 1024)], y[:])
                        else:
                            y = ypool.tile([128, 1024], BF16, name="yb",
                                           tag="yb", bufs=6)
                            nc.vector.tensor_scalar(
                                out=y[:], in0=src,
                                scalar1=scalev[:, ct:ct + 1],
                                scalar2=biasv[:, ct:ct + 1],
                                op0=OP.mult, op1=OP.add)
                            if k == 0:
                                nc.sync.dma_start(
                                    outy_d[ts(ct, 128), ts(h, 1024)], y[:])
                            elif k == 1:
                                nc.gpsimd.dma_start(
                                    outy_d[ts(ct, 128), ts(h, 1024)], y[:])
                            else:
                                nc.scalar.dma_start(
                                    outy_d[ts(ct, 128), ts(h, 1024)], y[:])

                for ct in range(NCC):
                    emit_outproj(ct)
                emit_stats_cc()
                emit_bn_params()
                for ct in range(NCC):
                    emit_apply(ct)

    nc.compile()
    return nc


def _get_compiled():
    global _compiled
    if _compiled is None:
        _compiled = _build()
    return _compiled


def kernel(x, Wt, Wp, Wg, Ww, gamma, beta, _trace=False, _trace_kwargs=None):
    global _last_results
    nc = _get_compiled()

    x = np.asarray(x, dtype=np.float32)
    Wt = np.asarray(Wt, dtype=np.float32)
    Wp = np.asarray(Wp, dtype=np.float32)
    Wg = np.asarray(Wg, dtype=np.float32)
    Ww = np.asarray(Ww, dtype=np.float32)
    gamma = np.asarray(gamma, dtype=np.float32)
    beta = np.asarray(beta, dtype=np.float32)

    bf = ml_dtypes.bfloat16
    f8e4 = ml_dtypes.float8_e4m3
    f8e3 = ml_dtypes.float8_e3m4

    def shuf(a):
        """[n*128, F] -> tile layout [128, n*F] (chunk-major free axis)."""
        n = a.shape[0] // 128
        return np.ascontiguousarray(
            a.reshape(n, 128, a.shape[1]).transpose(1, 0, 2).reshape(128, -1))

    def shuf_x(a):
        """[C, TOK] -> token-chunk-major tile layout [128, (tck, a, 512)]."""
        return np.ascontiguousarray(
            a.reshape(NCC, 128, NTC, 512).transpose(1, 2, 0, 3)
            .reshape(128, -1))

    wt_t = shuf((Wt.T * WG_SCALE).astype(f8e4))       # [C, L] -> tile
    wp_t = shuf(Wp.T.astype(bf))
    wg_t = shuf((Wg.T * WG_SCALE).astype(f8e4))
    ww_t = shuf((Ww.T * WW_SCALE).astype(f8e3))       # [L, C] -> tile
    r = np.arange(128)
    mask = (r[:, None] // T == r[None, :] // T).astype(bf)
    gb = np.concatenate(
        [gamma.reshape(NCC, 128).T,
         beta.reshape(NCC, 128).T], axis=1).astype(np.float32)  # [128, 16]

    # xf8/xf8b: actor-major token order (tok = j*T + t);
    # xbf: T-MAJOR token order (tok = t*N + j) to match the t-major stT
    # reads in the out-projection
    xa = x.transpose(0, 2, 1, 3).reshape(B, TOK, C)
    xt = x.reshape(B, TOK, C)                          # [B, (t n), C]
    in_maps = []
    for b in range(B):
        xT = np.ascontiguousarray(xa[b].T)            # [C, TOK] f32
        x8 = xT.astype(f8e4)
        xTt = np.ascontiguousarray(xt[b].T)           # [C, (t n)] f32
        in_maps.append(dict(
            xf8=shuf_x(x8), xf8b=shuf(x8), xbf=shuf_x(xTt.astype(bf)),
            wt=wt_t, wp=wp_t, wg=wg_t, ww=ww_t,
            mask=mask, gb=gb))

    res = run_bass_kernel_spmd(nc, in_maps, list(range(N_CORES)),
                               trace=_trace, **(_trace_kwargs or {}))
    _last_results = res

    ys = []
    for b in range(B):
        # outy cols are t-major: tok = t*N + j
        o = np.asarray(res.results[b]["outy"], dtype=np.float32)   # [C, TOK]
        ys.append(o.T.reshape(T, N, C))
    return np.stack(ys)


# revision 17
# speedup vs baseline: 1.4376x; 1.1468x over previous
"""Trainium2 Bass kernel for CrossInferBlock (spatial+temporal cross attention
+ out-projection + residual + BatchNorm over (B,T,N)).

Sharding: data-parallel over B across 8 NeuronCores (one batch element per
core). BN batch statistics are combined via an 8-core AllGather (8KB ->
64KB) + a local DVE reduce -- measured ~4x faster than AllReduce on this
fabric (AllReduce has a ~32us fixed cost; AllGather ~a third of that).

Precision plan (the residual dominates the output; the attention branch is
~17% of output magnitude, so fp8 there is cheap in accuracy; measured
rel err 1.6e-2 vs the fp32 reference, gate 2e-2):
  - theta + g projections: fp8e4 DoubleRow (2 K-tiles/pass = 2x PE
    throughput); x/Wt/Wg uploaded e4m3 (weights x16), outputs stored e3m4.
  - phi projection: bf16 (phi/theta errors multiply in the attention
    scores, so one of the pair stays high-precision).
  - attention scores tw/sw -> e3m4 (x1/32); applies tp/sp run fp8 at the
    bf16 rate; stT stored e3m4 (x8, max |stT| = 11.5 < 15.5).
  - out-projection: e3m4 x e3m4 (Ww x32); PSUM descaled by 2^-8 at the
    bf16 residual add. BN stats/apply in fp32.
All scale factors are powers of two (exact).

Device-side token order is ACTOR-MAJOR: tok = j*T + t. stT however is
stored T-MAJOR (free = lc*TOK + t*128 + j): the 16 per-timestep spatial
read-modify-write adds (phase 2, on the congested DVE) become contiguous
(~0.7us vs 2.6us strided), while the 16 temporal init writes eat the
stride on the half-idle ACT engine in phase 1. The out-projection reads
stT through a strided moving AP (stride-128 over t), which the PE walks
at full rate, so PSUM/residual/output stay token-major.

Phase order is chosen for DMA just-in-time: g_act+theta (needs only
wg/wt/xf8 = 3MB) start ~13us in while phi's inputs (wp/xbf = 5MB) and
the phase-2/3 tensors (xf8b, ww) stream in behind. theta and phi run
lc-major against 4 (2 for phi) concurrent PSUM accumulation groups so
each stationary weight tile is loaded once, not once per token chunk
(saves ~20k LDWEIGHTS columns). g_sp is precomputed in phase 1c so
phase 2 is only sw/sp + the cheap contiguous RMW.

Collectives: one warm-up AllGather at load time absorbs the CC stream's
one-time ~26us setup; a second keyed on phase-1c data keeps the ring
recent. The real stats AllGather fires as soon as the last out-projection
epilogue lands. BN apply+store is split across the DVE and ACT engines
with bf16 stores on three HWDGE rings (the host upcasts to fp32).
"""

import sys

if "/opt/trn_rl_repo" not in sys.path:
    sys.path.insert(0, "/opt/trn_rl_repo")

import numpy as np
import ml_dtypes

import concourse.bass as bass
import concourse.bacc as bacc
import concourse.tile as tile
import concourse.mybir as mybir
from concourse.bass_utils import run_bass_kernel_spmd
from contextlib import ExitStack

F32 = mybir.dt.float32
BF16 = mybir.dt.bfloat16
F8E4 = mybir.dt.float8e4     # e4m3: DoubleRow-capable
F8E3 = mybir.dt.float8e3     # e3m4: 2x mantissa, bf16-rate matmuls
AX = mybir.AxisListType
OP = mybir.AluOpType
ACT_FN = mybir.ActivationFunctionType
DR = mybir.MatmulPerfMode.DoubleRow

N_CORES = 8
B, T, N, C = 8, 16, 128, 1024
L = C // 2            # 512
TOK = T * N           # 2048 tokens per batch element
NTOK_GLOBAL = B * T * N
JG = 8                # actors per temporal group
NGRP = N // JG        # 16 groups
BN_EPS = 1e-5

WG_SCALE = 16.0       # Wg uploaded x16 (e4m3)
WW_SCALE = 32.0       # Ww uploaded x32 (e3m4: normal range starts at 0.25)
G_DESCALE = 1.0 / WG_SCALE
SB_SCALE = 1.0 / 32.0            # attention scores into e3m4 (std ~1.4)
STT_SCALE = 8.0                  # stT e3m4 boost (max|stT|=11.5 < 15.5)
SP_SCALE = STT_SCALE / (N * (T + N)) / SB_SCALE    # 0.027778
TP_SCALE = STT_SCALE / (T * (T + N)) / SB_SCALE    # 0.222
OUT_DESCALE = 1.0 / (STT_SCALE * WW_SCALE)         # 2^-9

NCC = C // 128     # 8 c-chunks
NLC = L // 128     # 4 l-chunks
NCP = NCC // 2     # 4 c-chunk pairs (DoubleRow)
NTC = TOK // 512   # 4 token chunks

_compiled = None
_last_results = None

USE_COLLECTIVE = True


def ts(i, size):
    return bass.ts(i, size)


def _build():
    nc = bacc.Bacc("TRN2", target_bir_lowering=False, debug=False,
                   num_devices=N_CORES)

    # ---- DRAM I/O (token order: actor-major, tok = j*T + t) ----
    # inputs are pre-shuffled on the host into the SBUF tile layout
    # [128, chunk*free] so every load is a full-row (4-32KB/row) DMA
    xf8_d = nc.dram_tensor("xf8", [128, NCC * TOK], F8E4,
                           kind="ExternalInput")
    # second fp8 x copy in channel-major layout: the spatial projection's
    # stride-T token gather needs (a, tok) order, which would make the
    # token-chunk-major tile a 4-free-dim DoubleRow weights AP
    xf8b_d = nc.dram_tensor("xf8b", [128, NCC * TOK], F8E4,
                            kind="ExternalInput")
    xbf_d = nc.dram_tensor("xbf", [128, NCC * TOK], BF16,
                           kind="ExternalInput")
    wt_d = nc.dram_tensor("wt", [128, NCC * L], F8E4, kind="ExternalInput")
    wp_d = nc.dram_tensor("wp", [128, NCC * L], BF16, kind="ExternalInput")
    wg_d = nc.dram_tensor("wg", [128, NCC * L], F8E4, kind="ExternalInput")
    ww_d = nc.dram_tensor("ww", [128, NLC * C], F8E3, kind="ExternalInput")
    mask_d = nc.dram_tensor("mask", [128, 128], BF16, kind="ExternalInput")
    gb_d = nc.dram_tensor("gb", [128, 16], F32, kind="ExternalInput")
    outy_d = nc.dram_tensor("outy", [C, TOK], BF16, kind="ExternalOutput")

    with tile.TileContext(nc) as tc:
        with ExitStack() as outer:
            # ---------------- persistent pools ----------------
            cpool = outer.enter_context(tc.tile_pool(name="consts", bufs=1))
            wwpool = outer.enter_context(tc.tile_pool(name="wwp", bufs=1))
            stpool = outer.enter_context(tc.tile_pool(name="stp", bufs=1))
            statpool = outer.enter_context(tc.tile_pool(name="stats", bufs=1))
            pbig = outer.enter_context(
                tc.tile_pool(name="pbig", bufs=1, space="PSUM"))
            psmall = outer.enter_context(
                tc.tile_pool(name="psmall", bufs=1, space="PSUM"))
            drampool = outer.enter_context(
                tc.tile_pool(name="dramp", bufs=1, space="DRAM"))
            xbpool = outer.enter_context(tc.tile_pool(name="xbp", bufs=1))

            mask_sb = cpool.tile([128, 128], BF16, name="mask_sb",
                                 tag="mask_sb")
            gb_sb = cpool.tile([128, 16], F32, name="gb_sb", tag="gb_sb")
            ww_all = wwpool.tile([128, NLC * C], F8E3, name="ww_all", tag="ww")
            # stT is T-MAJOR: free = lc*TOK + t*128 + j
            stT = stpool.tile([128, NLC * TOK], F8E3, name="stT", tag="stT")

            stat_sum = statpool.tile([128, 32], F32, name="stat_sum",
                                     tag="stat_sum")
            stat_sq = statpool.tile([128, 32], F32, name="stat_sq",
                                    tag="stat_sq")
            red_in = statpool.tile([128, 16], F32, name="red_in", tag="red_in")
            ag_sb = statpool.tile([128, N_CORES * 16], F32, name="ag_sb",
                                  tag="ag_sb")
            red_out = statpool.tile([128, 16], F32, name="red_out",
                                    tag="red_out")
            scalev = statpool.tile([128, 8], F32, name="scalev", tag="scalev")
            biasv = statpool.tile([128, 8], F32, name="biasv", tag="biasv")

            cc_warm_in = drampool.tile([128, 1], F32, name="cc_warm_in",
                                       tag="cc_warm_in")
            cc_warm_out = drampool.tile([N_CORES * 128, 1], F32,
                                        name="cc_warm_out", tag="cc_warm_out")
            cc_w2_in = drampool.tile([128, 1], F32, name="cc_w2_in",
                                     tag="cc_w2_in")
            cc_w2_out = drampool.tile([N_CORES * 128, 1], F32,
                                      name="cc_w2_out", tag="cc_w2_out")

            with ExitStack() as mid:
                thpool = mid.enter_context(tc.tile_pool(name="thp", bufs=1))
                gpool = mid.enter_context(tc.tile_pool(name="gp", bufs=1))
                attnpool = mid.enter_context(tc.tile_pool(name="attn", bufs=1))

                thT = thpool.tile([128, NLC * TOK], F8E3, name="thT",
                                  tag="thT")
                phT = thpool.tile([128, NLC * TOK], F8E3, name="phT",
                                  tag="phT")
                g_sp = [gpool.tile([128, L], F8E3, name=f"gsp{i}",
                                   tag=f"gsp{i}") for i in range(T)]
                g_act = [gpool.tile([128, L], F8E3, name=f"gact{j}",
                                    tag=f"gact{j}") for j in range(NGRP)]

                with ExitStack() as phase_a:
                    wpool = phase_a.enter_context(
                        tc.tile_pool(name="wp", bufs=1))

                    xf8 = xbpool.tile([128, NCC * TOK], F8E4, name="xf8",
                                      tag="xf8")
                    xf8b = xbpool.tile([128, NCC * TOK], F8E4, name="xf8b",
                                       tag="xf8b")
                    xbf = xbpool.tile([128, NCC * TOK], BF16, name="xbf",
                                      tag="xbf")
                    wt_all = wpool.tile([128, NCC * L], F8E4, name="wt_all",
                                        tag="wt")
                    wp_all = wpool.tile([128, NCC * L], BF16, name="wp_all",
                                        tag="wp")
                    wg_all = wpool.tile([128, NCC * L], F8E4, name="wg_all",
                                        tag="wg")

                    # input DMA schedule. There are TWO effective input
                    # pipes: the sync HWDGE queue, and a second HWDGE queue
                    # SHARED by the gpsimd and scalar engines (their
                    # descriptors interleave). Critical phase-1a tensors
                    # (wg halves, xf8 chunks, wt) ride the front of both
                    # pipes; phi inputs (wp/xbf) follow; xf8b (1c) and ww
                    # (phase 3) last.
                    CHW = NCC * 512          # flat cols per token chunk
                    HW = NCC * L // 2        # half of a weight tile
                    nc.sync.dma_start(xf8[:, 0:CHW], xf8_d[:, 0:CHW])
                    nc.gpsimd.dma_start(wg_all[:, 0:HW], wg_d[:, 0:HW])
                    nc.scalar.dma_start(xf8[:, ts(1, CHW)],
                                        xf8_d[:, ts(1, CHW)])
                    nc.sync.dma_start(wg_all[:, HW:2 * HW],
                                      wg_d[:, HW:2 * HW])
                    nc.gpsimd.dma_start(xf8[:, ts(2, CHW)],
                                        xf8_d[:, ts(2, CHW)])
                    nc.scalar.dma_start(xf8[:, ts(3, CHW)],
                                        xf8_d[:, ts(3, CHW)])
                    nc.sync.dma_start(wt_all[:], wt_d[:])
                    nc.gpsimd.dma_start(wp_all[:], wp_d[:])
                    nc.sync.dma_start(xbf[:, 0:CHW], xbf_d[:, 0:CHW])
                    nc.scalar.dma_start(xbf[:, ts(2, CHW)],
                                        xbf_d[:, ts(2, CHW)])
                    nc.sync.dma_start(xbf[:, ts(1, CHW)],
                                      xbf_d[:, ts(1, CHW)])
                    nc.gpsimd.dma_start(xbf[:, ts(3, CHW)],
                                        xbf_d[:, ts(3, CHW)])
                    nc.gpsimd.dma_start(mask_sb[:], mask_d[:])
                    nc.gpsimd.dma_start(gb_sb[:], gb_d[:])
                    nc.scalar.dma_start(xf8b[:], xf8b_d[:])
                    nc.sync.dma_start(ww_all[:], ww_d[:])
                    if USE_COLLECTIVE:
                        # warm-up collective #1: pays the CC stream's
                        # one-time setup during the DMA load
                        nc.gpsimd.dma_start(cc_warm_in[:], gb_d[:, 0:1])
                        nc.gpsimd.collective_compute(
                            "AllGather", OP.bypass,
                            replica_groups=[list(range(N_CORES))],
                            ins=[cc_warm_in.opt()], outs=[cc_warm_out.opt()])

                    # views (x tiles are token-chunk-major: (tck, a, k))
                    xv8 = xf8.rearrange("p (tk a k) -> p tk a k",
                                        tk=NTC, a=NCC)
                    # spatial: tok = j*T + t (channel-major copy)
                    xsp8 = xf8b.rearrange("p (a j t) -> p a t j",
                                          a=NCC, t=T)
                    xbv = xbf.rearrange("p (tk a k) -> p tk a k",
                                        tk=NTC, a=NCC)
                    wgv = wg_all.rearrange("p (a l) -> p a l", a=NCC)
                    wtv = wt_all.rearrange("p (a l) -> p a l", a=NCC)

                    def xsl(c, tck):
                        return xbv[:, tck, c, :]

                    def wsl(w, c, lc):
                        return w[:, c * L + lc * 128:c * L + (lc + 1) * 128]

                    # ------- phase 1a: g_act (xf8+wg) then theta (wt) ------
                    for jg in range(NGRP):
                        tck = jg // 4
                        ps = pbig.tile([128, 512], F32, name="ps_ga",
                                       tag="ps_big", bufs=4)
                        for cp in range(NCP):
                            nc.tensor.matmul(
                                ps[:],
                                xv8[:, tck, 2 * cp:2 * cp + 2,
                                    ts(jg - 4 * tck, 128)],
                                wgv[:, 2 * cp:2 * cp + 2, :],
                                start=(cp == 0), stop=(cp == NCP - 1),
                                perf_mode=DR)
                        nc.scalar.mul(g_act[jg][:], ps[:], G_DESCALE)

                    # theta: fp8 DoubleRow, lc-major with 4 concurrent tck
                    # PSUM groups -- each wt tile is loaded once, serving
                    # 4 back-to-back matmuls (consecutive loads dedupe)
                    for lc in range(NLC):
                        pss = [pbig.tile([128, 512], F32, name=f"ps_th{t}",
                                         tag="ps_big", bufs=4)
                               for t in range(NTC)]
                        for cp in range(NCP):
                            for tck in range(NTC):
                                nc.tensor.matmul(
                                    pss[tck][:],
                                    wtv[:, 2 * cp:2 * cp + 2, ts(lc, 128)],
                                    xv8[:, tck, 2 * cp:2 * cp + 2, :],
                                    start=(cp == 0), stop=(cp == NCP - 1),
                                    perf_mode=DR)
                        for tck in range(NTC):
                            dst = thT[:, lc * TOK + tck * 512:
                                      lc * TOK + tck * 512 + 512]
                            nc.vector.tensor_scalar_mul(dst, pss[tck][:],
                                                        G_DESCALE)

                    # ------- phase 1b: phi (bf16; wp + xbf) ---------------
                    # xbf is T-MAJOR (tq = t//4 quarter chunks) so the
                    # out-projection's residual add matches the contiguous
                    # t-major stT reads; phi's PSUM therefore comes out
                    # t-major and is scattered into the actor-major phT
                    # (stride-16 writes, alternating DVE/ACT -- both
                    # half-idle here, hidden under phi's 36us of matmul).
                    # lc-major over tq pairs (2 concurrent PSUM groups) so
                    # phi can start once xbf chunks 0-1 have landed.
                    phTv = phT.rearrange("p (a j t) -> p a t j",
                                         a=NLC, t=T)
                    for half in range(2):
                        tqs = (2 * half, 2 * half + 1)
                        for lc in range(NLC):
                            pss = [pbig.tile([128, 512], F32,
                                             name=f"ps_ph{t}",
                                             tag="ps_big", bufs=4)
                                   for t in tqs]
                            for c in range(NCC):
                                for k, tq in enumerate(tqs):
                                    nc.tensor.matmul(
                                        pss[k][:], wsl(wp_all, c, lc),
                                        xsl(c, tq),
                                        start=(c == 0), stop=(c == NCC - 1))
                            for k, tq in enumerate(tqs):
                                dst = phTv[:, lc, 4 * tq:4 * tq + 4, :]
                                src = pss[k].rearrange("p (t j) -> p t j",
                                                       t=4)
                                if lc % 2 == 0:
                                    nc.vector.tensor_copy(dst, src)
                                else:
                                    nc.scalar.copy(dst, src)

                    # ---- phase 1c: temporal attention + g_sp precompute --
                    # temporal INITIALIZES stT (t-major, strided write on the
                    # half-idle ACT engine); g_sp precomputed here so phase 2
                    # is only sw/sp
                    pend_tp = []   # (jg, twp)

                    def emit_tw(jg):
                        twp = psmall.tile([128, 128], F32, name="ps_tw",
                                          tag="ps_small", bufs=4)
                        for lc in range(NLC):
                            nc.tensor.matmul(
                                twp[:],
                                phT[:, lc * TOK + jg * 128:
                                    lc * TOK + jg * 128 + 128],
                                thT[:, lc * TOK + jg * 128:
                                    lc * TOK + jg * 128 + 128],
                                start=(lc == 0), stop=(lc == NLC - 1))
                        pend_tp.append((jg, twp))

                    def emit_tp():
                        jg, twp = pend_tp.pop(0)
                        sb = attnpool.tile([128, 128], F8E3, name="sb",
                                           tag="sb", bufs=3)
                        nc.vector.scalar_tensor_tensor(
                            out=sb[:], in0=twp[:], scalar=SB_SCALE,
                            in1=mask_sb[:], op0=OP.mult, op1=OP.mult)
                        pp = psmall.tile([128, 512], F32, name="ps_tp",
                                         tag="ps_small", bufs=4)
                        for lc in range(NLC):
                            nc.tensor.matmul(pp[:, ts(lc, 128)],
                                             g_act[jg][:, ts(lc, 128)], sb[:])
                        # pp free = (lc, j8, t16); stT t-major dst
                        # free = lc*2048 + t*128 + (8*jg + j)
                        dst = stT.rearrange("p (a t j) -> p a t j",
                                            a=NLC, t=T)[
                            :, :, :, ts(jg, JG)]
                        src = pp.rearrange("p (a j t) -> p a t j",
                                           a=NLC, j=JG)
                        nc.scalar.mul(dst, src, TP_SCALE)

                    def emit_gsp(i):
                        ps = pbig.tile([128, 512], F32, name="ps_g",
                                       tag="ps_big", bufs=4)
                        for cp in range(NCP):
                            nc.tensor.matmul(
                                ps[:],
                                xsp8[:, 2 * cp:2 * cp + 2, i:i + 1, :],
                                wgv[:, 2 * cp:2 * cp + 2, :],
                                start=(cp == 0), stop=(cp == NCP - 1),
                                perf_mode=DR)
                        nc.scalar.mul(g_sp[i][:], ps[:], G_DESCALE)

                    first_done = False
                    for k in range(NGRP):
                        emit_gsp(k)
                        emit_tw(k)
                        if not first_done and USE_COLLECTIVE:
                            # warm-up collective #2, keyed on 1c data so the
                            # CC stream is recently-used when the real stats
                            # collective triggers
                            nc.gpsimd.dma_start(cc_w2_in[:],
                                                g_sp[0][:, 0:1])
                            nc.gpsimd.collective_compute(
                                "AllGather", OP.bypass,
                                replica_groups=[list(range(N_CORES))],
                                ins=[cc_w2_in.opt()], outs=[cc_w2_out.opt()])
                            first_done = True
                        if len(pend_tp) >= 2:
                            emit_tp()
                    while pend_tp:
                        emit_tp()

                    # ------- phase 2: spatial attention (ADD into stT) ----
                    pend_sp = []   # (i, swp)

                    def sp_view(tile_ap, i):
                        return tile_ap.rearrange(
                            "p (j t) -> p t j", t=T)[:, i:i + 1, :]

                    def thsl(tt, lc):
                        return tt[:, lc * TOK:(lc + 1) * TOK]

                    def emit_sw(i):
                        swp = psmall.tile([128, 128], F32, name="ps_sw",
                                          tag="ps_small", bufs=4)
                        for lc in range(NLC):
                            nc.tensor.matmul(swp[:],
                                             sp_view(thsl(phT, lc), i),
                                             sp_view(thsl(thT, lc), i),
                                             start=(lc == 0),
                                             stop=(lc == NLC - 1))
                        pend_sp.append((i, swp))

                    def emit_sp():
                        i, swp = pend_sp.pop(0)
                        swb = attnpool.tile([128, 128], F8E3, name="swb",
                                            tag="swb", bufs=3)
                        nc.scalar.mul(swb[:], swp[:], SB_SCALE)
                        pp = psmall.tile([128, 512], F32, name="ps_sp",
                                         tag="ps_small", bufs=4)
                        for lc in range(NLC):
                            nc.tensor.matmul(pp[:, ts(lc, 128)],
                                             g_sp[i][:, ts(lc, 128)], swb[:])
                        # t-major stT: the t=i row (all 128 actors) is a
                        # contiguous 128-run per lc chunk -> fast DVE RMW
                        dst = stT.rearrange("p (a t j) -> p a t j",
                                            a=NLC, t=T)[:, :, i, :]
                        src = pp.rearrange("p (a j) -> p a j", a=NLC)
                        nc.vector.scalar_tensor_tensor(
                            out=dst, in0=src, scalar=SP_SCALE, in1=dst,
                            op0=OP.mult, op1=OP.add)

                    for i in range(T):
                        emit_sw(i)
                        if len(pend_sp) >= 2:
                            emit_sp()
                    while pend_sp:
                        emit_sp()

            # ------- phase 3: out-projection + residual + stats -------
            with tc.tile_pool(name="outp", bufs=1) as outpool, \
                 tc.tile_pool(name="yp", bufs=1) as ypool, \
                 tc.tile_pool(name="sqp", bufs=1) as sqpool:
                out_sb = []
                inv_n = 1.0 / float(NTOK_GLOBAL)

                def emit_outproj(ct):
                    o = outpool.tile([128, TOK], BF16, name=f"out{ct}",
                                     tag=f"out{ct}")
                    out_sb.append(o)
                    # tq-inner with 4 concurrent PSUM groups: each ww
                    # weight tile serves 4 back-to-back matmuls; the moving
                    # operand is a CONTIGUOUS 512-col t-major stT slice
                    # (tokens t in [4tq, 4tq+4), all actors), matching the
                    # t-major xbf/output layout
                    pss = [pbig.tile([128, 512], F32, name=f"ps_out{t}",
                                     tag="ps_big", bufs=4)
                           for t in range(NTC)]
                    for lc in range(NLC):
                        for tq in range(NTC):
                            nc.tensor.matmul(
                                pss[tq][:],
                                ww_all[:, lc * C + ct * 128:
                                       lc * C + (ct + 1) * 128],
                                stT[:, lc * TOK + tq * 512:
                                    lc * TOK + tq * 512 + 512],
                                start=(lc == 0), stop=(lc == NLC - 1))
                    for tq in range(NTC):
                        col = ct * NTC + tq
                        nc.vector.scalar_tensor_tensor(
                            out=o[:, ts(tq, 512)], in0=pss[tq][:],
                            scalar=OUT_DESCALE,
                            in1=xbf.rearrange("p (tk a k) -> p tk a k",
                                              tk=NTC, a=NCC)[:, tq, ct, :],
                            op0=OP.mult, op1=OP.add,
                            accum_out=stat_sum[:, col:col + 1])
                        sq = sqpool.tile([128, 512], F32, name="sqscr",
                                         tag="sq", bufs=3)
                        nc.scalar.activation(
                            sq[:], o[:, ts(tq, 512)], ACT_FN.Square,
                            accum_out=stat_sq[:, col:col + 1])

                # stats are collected and all-gathered in TWO ct-halves:
                # the first AllGather (channels 0-511) fires as soon as
                # out-projection chunks 0-3 land and completes under the
                # remaining chunks' compute; only the second (tiny) AG's
                # ~5us latency is exposed, and the first half's BN params +
                # applies + stores overlap it.
                cc_h_in = [drampool.tile([128, 8], F32, name=f"cc_in{g}",
                                         tag=f"cc_in{g}") for g in range(2)]
                cc_h_out = [drampool.tile([N_CORES * 128, 8], F32,
                                          name=f"cc_out{g}",
                                          tag=f"cc_out{g}") for g in range(2)]

                def emit_stats_cc(g):
                    """AllGather per-channel sum/sumsq for ct in
                    [4g, 4g+4) + local cross-core reduce."""
                    nc.vector.tensor_reduce(
                        red_in[:, 8 * g:8 * g + 4],
                        stat_sum.rearrange("p (a b) -> p a b",
                                           a=8)[:, 4 * g:4 * g + 4, :],
                        axis=AX.X, op=OP.add)
                    nc.vector.tensor_reduce(
                        red_in[:, 8 * g + 4:8 * g + 8],
                        stat_sq.rearrange("p (a b) -> p a b",
                                          a=8)[:, 4 * g:4 * g + 4, :],
                        axis=AX.X, op=OP.add)
                    if USE_COLLECTIVE:
                        nc.gpsimd.dma_start(cc_h_in[g][:],
                                            red_in[:, 8 * g:8 * g + 8])
                        nc.gpsimd.collective_compute(
                            "AllGather", OP.bypass,
                            replica_groups=[list(range(N_CORES))],
                            ins=[cc_h_in[g].opt()],
                            outs=[cc_h_out[g].opt()])
                        src = cc_h_out[g].rearrange("(r p) c -> p r c",
                                                    r=N_CORES)
                        dst = ag_sb.rearrange("p (g r c) -> p g r c",
                                              g=2, r=N_CORES)[:, g]
                        nc.gpsimd.dma_start(dst, src)
                        agv = ag_sb.rearrange("p (g r c) -> p g c r",
                                              g=2, r=N_CORES)
                        # cols 0:4 are sums, 4:8 sumsq for this half
                        nc.vector.tensor_reduce(
                            red_out[:, 4 * g:4 * g + 4],
                            agv[:, g, 0:4, :], axis=AX.X, op=OP.add)
                        nc.vector.tensor_reduce(
                            red_out[:, 8 + 4 * g:8 + 4 * g + 4],
                            agv[:, g, 4:8, :], axis=AX.X, op=OP.add)
                    else:
                        nc.vector.tensor_scalar_mul(
                            red_out[:, 4 * g:4 * g + 4],
                            red_in[:, 8 * g:8 * g + 4], float(N_CORES))
                        nc.vector.tensor_scalar_mul(
                            red_out[:, 8 + 4 * g:8 + 4 * g + 4],
                            red_in[:, 8 * g + 4:8 * g + 8], float(N_CORES))

                def emit_bn_params(g):
                    lo, hi = 4 * g, 4 * g + 4
                    mean = statpool.tile([128, 4], F32, name=f"mean{g}",
                                         tag=f"mean{g}")
                    var = statpool.tile([128, 4], F32, name=f"var{g}",
                                        tag=f"var{g}")
                    std = statpool.tile([128, 4], F32, name=f"std{g}",
                                        tag=f"std{g}")
                    rstd = statpool.tile([128, 4], F32, name=f"rstd{g}",
                                         tag=f"rstd{g}")
                    nc.vector.tensor_scalar_mul(mean[:], red_out[:, lo:hi],
                                                inv_n)
                    nc.vector.tensor_scalar_mul(var[:],
                                                red_out[:, 8 + lo:8 + hi],
                                                inv_n)
                    nc.vector.tensor_mul(std[:], mean[:], mean[:])
                    nc.vector.tensor_tensor(var[:], var[:], std[:],
                                            op=OP.subtract)
                    nc.vector.tensor_scalar_add(var[:], var[:], BN_EPS)
                    nc.scalar.activation(std[:], var[:], ACT_FN.Sqrt, bias=0.0)
                    nc.vector.reciprocal(rstd[:], std[:])
                    nc.vector.tensor_mul(scalev[:, lo:hi], rstd[:],
                                         gb_sb[:, lo:hi])
                    nc.vector.tensor_mul(rstd[:], mean[:], scalev[:, lo:hi])
                    nc.vector.tensor_tensor(biasv[:, lo:hi],
                                            gb_sb[:, 8 + lo:8 + hi], rstd[:],
                                            op=OP.subtract)

                def emit_apply(ct):
                    # DVE is ~2.3x faster per op here than ACT: give DVE 13
                    # of 16 half-tiles, ACT 3; stores round-robin the sync,
                    # gpsimd and scalar rings; bf16 stores (host upcasts)
                    for h in range(2):
                        i = 2 * ct + h
                        src = out_sb[ct][:, ts(h, 1024)]
                        if i % 5 == 4:
                            y = ypool.tile([128, 1024], BF16, name="ya",
                                           tag="ya", bufs=4)
                            nc.scalar.activation(
                                y[:], src, ACT_FN.Identity,
                                scale=scalev[:, ct:ct + 1],
                                bias=biasv[:, ct:ct + 1])
                        else:
                            y = ypool.tile([128, 1024], BF16, name="yb",
                                           tag="yb", bufs=8)
                            nc.vector.tensor_scalar(
                                out=y[:], in0=src,
                                scalar1=scalev[:, ct:ct + 1],
                                scalar2=biasv[:, ct:ct + 1],
                                op0=OP.mult, op1=OP.add)
                        if i % 3 == 0:
                            nc.sync.dma_start(
                                outy_d[ts(ct, 128), ts(h, 1024)], y[:])
                        elif i % 3 == 1:
                            nc.gpsimd.dma_start(
                                outy_d[ts(ct, 128), ts(h, 1024)], y[:])
                        else:
                            nc.scalar.dma_start(
                                outy_d[ts(ct, 128), ts(h, 1024)], y[:])

                for ct in range(4):
                    emit_outproj(ct)
                emit_stats_cc(0)
                for ct in range(4, NCC):
                    emit_outproj(ct)
                emit_stats_cc(1)
                emit_bn_params(0)
                for ct in range(4):
                    emit_apply(ct)
                emit_bn_params(1)
                for ct in range(4, NCC):
                    emit_apply(ct)

    nc.compile()
    return nc


def _get_compiled():
    global _compiled
    if _compiled is None:
        _compiled = _build()
    return _compiled


def kernel(x, Wt, Wp, Wg, Ww, gamma, beta, _trace=False, _trace_kwargs=None):
    global _last_results
    nc = _get_compiled()

    x = np.asarray(x, dtype=np.float32)
    Wt = np.asarray(Wt, dtype=np.float32)
    Wp = np.asarray(Wp, dtype=np.float32)
    Wg = np.asarray(Wg, dtype=np.float32)
    Ww = np.asarray(Ww, dtype=np.float32)
    gamma = np.asarray(gamma, dtype=np.float32)
    beta = np.asarray(beta, dtype=np.float32)

    bf = ml_dtypes.bfloat16
    f8e4 = ml_dtypes.float8_e4m3
    f8e3 = ml_dtypes.float8_e3m4

    def shuf(a):
        """[n*128, F] -> tile layout [128, n*F] (chunk-major free axis)."""
        n = a.shape[0] // 128
        return np.ascontiguousarray(
            a.reshape(n, 128, a.shape[1]).transpose(1, 0, 2).reshape(128, -1))

    def shuf_x(a):
        """[C, TOK] -> token-chunk-major tile layout [128, (tck, a, 512)]."""
        return np.ascontiguousarray(
            a.reshape(NCC, 128, NTC, 512).transpose(1, 2, 0, 3)
            .reshape(128, -1))

    wt_t = shuf((Wt.T * WG_SCALE).astype(f8e4))       # [C, L] -> tile
    wp_t = shuf(Wp.T.astype(bf))
    wg_t = shuf((Wg.T * WG_SCALE).astype(f8e4))
    ww_t = shuf((Ww.T * WW_SCALE).astype(f8e3))       # [L, C] -> tile
    r = np.arange(128)
    mask = (r[:, None] // T == r[None, :] // T).astype(bf)
    gb = np.concatenate(
        [gamma.reshape(NCC, 128).T,
         beta.reshape(NCC, 128).T], axis=1).astype(np.float32)  # [128, 16]

    # xf8/xf8b: actor-major token order (tok = j*T + t);
    # xbf: T-MAJOR token order (tok = t*N + j) to match the t-major stT
    # reads in the out-projection
    xa = x.transpose(0, 2, 1, 3).reshape(B, TOK, C)
    xt = x.reshape(B, TOK, C)                          # [B, (t n), C]
    in_maps = []
    for b in range(B):
        xT = np.ascontiguousarray(xa[b].T)            # [C, TOK] f32
        x8 = xT.astype(f8e4)
        xTt = np.ascontiguousarray(xt[b].T)           # [C, (t n)] f32
        in_maps.append(dict(
            xf8=shuf_x(x8), xf8b=shuf(x8), xbf=shuf_x(xTt.astype(bf)),
            wt=wt_t, wp=wp_t, wg=wg_t, ww=ww_t,
            mask=mask, gb=gb))

    res = run_bass_kernel_spmd(nc, in_maps, list(range(N_CORES)),
                               trace=_trace, **(_trace_kwargs or {}))
    _last_results = res

    ys = []
    for b in range(B):
        # outy cols are t-major: tok = t*N + j
        o = np.asarray(res.results[b]["outy"], dtype=np.float32)   # [C, TOK]
        ys.append(o.T.reshape(T, N, C))
    return np.stack(ys)


# revision 21
# speedup vs baseline: 1.4462x; 1.0060x over previous
"""Trainium2 Bass kernel for CrossInferBlock (spatial+temporal cross attention
+ out-projection + residual + BatchNorm over (B,T,N)).

Sharding: data-parallel over B across 8 NeuronCores (one batch element per
core). BN batch statistics are combined via an 8-core AllGather (8KB ->
64KB) + a local DVE reduce -- measured ~4x faster than AllReduce on this
fabric (AllReduce has a ~32us fixed cost; AllGather ~a third of that).

Precision plan (the residual dominates the output; the attention branch is
~17% of output magnitude, so fp8 there is cheap in accuracy; measured
rel err 1.6e-2 vs the fp32 reference, gate 2e-2):
  - theta + g projections: fp8e4 DoubleRow (2 K-tiles/pass = 2x PE
    throughput); x/Wt/Wg uploaded e4m3 (weights x16), outputs stored e3m4.
  - phi projection: bf16 (phi/theta errors multiply in the attention
    scores, so one of the pair stays high-precision).
  - attention scores tw/sw -> e3m4 (x1/32); applies tp/sp run fp8 at the
    bf16 rate; stT stored e3m4 (x8, max |stT| = 11.5 < 15.5).
  - out-projection: e3m4 x e3m4 (Ww x32); PSUM descaled by 2^-8 at the
    bf16 residual add. BN stats/apply in fp32.
All scale factors are powers of two (exact).

Device-side token order is ACTOR-MAJOR: tok = j*T + t. stT however is
stored T-MAJOR (free = lc*TOK + t*128 + j): the 16 per-timestep spatial
read-modify-write adds (phase 2, on the congested DVE) become contiguous
(~0.7us vs 2.6us strided), while the 16 temporal init writes eat the
stride on the half-idle ACT engine in phase 1. The out-projection reads
stT through a strided moving AP (stride-128 over t), which the PE walks
at full rate, so PSUM/residual/output stay token-major.

Phase order is chosen for DMA just-in-time: g_act+theta (needs only
wg/wt/xf8 = 3MB) start ~13us in while phi's inputs (wp/xbf = 5MB) and
the phase-2/3 tensors (xf8b, ww) stream in behind. theta and phi run
lc-major against 4 (2 for phi) concurrent PSUM accumulation groups so
each stationary weight tile is loaded once, not once per token chunk
(saves ~20k LDWEIGHTS columns). g_sp is precomputed in phase 1c so
phase 2 is only sw/sp + the cheap contiguous RMW.

Collectives: one warm-up AllGather at load time absorbs the CC stream's
one-time ~26us setup; a second keyed on phase-1c data keeps the ring
recent. The real stats AllGather fires as soon as the last out-projection
epilogue lands. BN apply+store is split across the DVE and ACT engines
with bf16 stores on three HWDGE rings (the host upcasts to fp32).
"""

import sys

if "/opt/trn_rl_repo" not in sys.path:
    sys.path.insert(0, "/opt/trn_rl_repo")

import numpy as np
import ml_dtypes

import concourse.bass as bass
import concourse.bacc as bacc
import concourse.tile as tile
import concourse.mybir as mybir
from concourse.bass_utils import run_bass_kernel_spmd
from contextlib import ExitStack

F32 = mybir.dt.float32
BF16 = mybir.dt.bfloat16
F8E4 = mybir.dt.float8e4     # e4m3: DoubleRow-capable
F8E3 = mybir.dt.float8e3     # e3m4: 2x mantissa, bf16-rate matmuls
AX = mybir.AxisListType
OP = mybir.AluOpType
ACT_FN = mybir.ActivationFunctionType
DR = mybir.MatmulPerfMode.DoubleRow

N_CORES = 8
B, T, N, C = 8, 16, 128, 1024
L = C // 2            # 512
TOK = T * N           # 2048 tokens per batch element
NTOK_GLOBAL = B * T * N
JG = 8                # actors per temporal group
NGRP = N // JG        # 16 groups
BN_EPS = 1e-5

WG_SCALE = 16.0       # Wg uploaded x16 (e4m3)
WW_SCALE = 32.0       # Ww uploaded x32 (e3m4: normal range starts at 0.25)
G_DESCALE = 1.0 / WG_SCALE
SB_SCALE = 1.0 / 32.0            # attention scores into e3m4 (std ~1.4)
STT_SCALE = 8.0                  # stT e3m4 boost (max|stT|=11.5 < 15.5)
SP_SCALE = STT_SCALE / (N * (T + N)) / SB_SCALE    # 0.027778
TP_SCALE = STT_SCALE / (T * (T + N)) / SB_SCALE    # 0.222
OUT_DESCALE = 1.0 / (STT_SCALE * WW_SCALE)         # 2^-9

NCC = C // 128     # 8 c-chunks
NLC = L // 128     # 4 l-chunks
NCP = NCC // 2     # 4 c-chunk pairs (DoubleRow)
NTC = TOK // 512   # 4 token chunks

_compiled = None
_last_results = None

USE_COLLECTIVE = True


def ts(i, size):
    return bass.ts(i, size)


def _build():
    nc = bacc.Bacc("TRN2", target_bir_lowering=False, debug=False,
                   num_devices=N_CORES)

    # ---- DRAM I/O (token order: actor-major, tok = j*T + t) ----
    # inputs are pre-shuffled on the host into the SBUF tile layout
    # [128, chunk*free] so every load is a full-row (4-32KB/row) DMA
    xf8_d = nc.dram_tensor("xf8", [128, NCC * TOK], F8E4,
                           kind="ExternalInput")
    # second fp8 x copy in channel-major layout: the spatial projection's
    # stride-T token gather needs (a, tok) order, which would make the
    # token-chunk-major tile a 4-free-dim DoubleRow weights AP
    xf8b_d = nc.dram_tensor("xf8b", [128, NCC * TOK], F8E4,
                            kind="ExternalInput")
    xbf_d = nc.dram_tensor("xbf", [128, NCC * TOK], BF16,
                           kind="ExternalInput")
    wt_d = nc.dram_tensor("wt", [128, NCC * L], F8E4, kind="ExternalInput")
    wp_d = nc.dram_tensor("wp", [128, NCC * L], BF16, kind="ExternalInput")
    wg_d = nc.dram_tensor("wg", [128, NCC * L], F8E4, kind="ExternalInput")
    ww_d = nc.dram_tensor("ww", [128, NLC * C], F8E3, kind="ExternalInput")
    mask_d = nc.dram_tensor("mask", [128, 128], BF16, kind="ExternalInput")
    gb_d = nc.dram_tensor("gb", [128, 16], F32, kind="ExternalInput")
    outy_d = nc.dram_tensor("outy", [C, TOK], BF16, kind="ExternalOutput")

    with tile.TileContext(nc) as tc:
        with ExitStack() as outer:
            # ---------------- persistent pools ----------------
            cpool = outer.enter_context(tc.tile_pool(name="consts", bufs=1))
            wwpool = outer.enter_context(tc.tile_pool(name="wwp", bufs=1))
            stpool = outer.enter_context(tc.tile_pool(name="stp", bufs=1))
            statpool = outer.enter_context(tc.tile_pool(name="stats", bufs=1))
            pbig = outer.enter_context(
                tc.tile_pool(name="pbig", bufs=1, space="PSUM"))
            psmall = outer.enter_context(
                tc.tile_pool(name="psmall", bufs=1, space="PSUM"))
            drampool = outer.enter_context(
                tc.tile_pool(name="dramp", bufs=1, space="DRAM"))
            xbpool = outer.enter_context(tc.tile_pool(name="xbp", bufs=1))

            mask_sb = cpool.tile([128, 128], BF16, name="mask_sb",
                                 tag="mask_sb")
            gb_sb = cpool.tile([128, 16], F32, name="gb_sb", tag="gb_sb")
            ww_all = wwpool.tile([128, NLC * C], F8E3, name="ww_all", tag="ww")
            # stT is T-MAJOR: free = lc*TOK + t*128 + j
            stT = stpool.tile([128, NLC * TOK], F8E3, name="stT", tag="stT")

            stat_sum = statpool.tile([128, 32], F32, name="stat_sum",
                                     tag="stat_sum")
            stat_sq = statpool.tile([128, 8], F32, name="stat_sq",
                                    tag="stat_sq")
            red_in = statpool.tile([128, 16], F32, name="red_in", tag="red_in")
            ag_sb = statpool.tile([128, N_CORES * 16], F32, name="ag_sb",
                                  tag="ag_sb")
            red_out = statpool.tile([128, 16], F32, name="red_out",
                                    tag="red_out")
            scalev = statpool.tile([128, 8], F32, name="scalev", tag="scalev")
            biasv = statpool.tile([128, 8], F32, name="biasv", tag="biasv")

            cc_warm_in = drampool.tile([128, 1], F32, name="cc_warm_in",
                                       tag="cc_warm_in")
            cc_warm_out = drampool.tile([N_CORES * 128, 1], F32,
                                        name="cc_warm_out", tag="cc_warm_out")
            cc_w2_in = drampool.tile([128, 1], F32, name="cc_w2_in",
                                     tag="cc_w2_in")
            cc_w2_out = drampool.tile([N_CORES * 128, 1], F32,
                                      name="cc_w2_out", tag="cc_w2_out")

            with ExitStack() as mid:
                thpool = mid.enter_context(tc.tile_pool(name="thp", bufs=1))
                gpool = mid.enter_context(tc.tile_pool(name="gp", bufs=1))
                attnpool = mid.enter_context(tc.tile_pool(name="attn", bufs=1))

                thT = thpool.tile([128, NLC * TOK], F8E3, name="thT",
                                  tag="thT")
                phT = thpool.tile([128, NLC * TOK], F8E3, name="phT",
                                  tag="phT")
                g_sp = [gpool.tile([128, L], F8E3, name=f"gsp{i}",
                                   tag=f"gsp{i}") for i in range(T)]
                g_act = [gpool.tile([128, L], F8E3, name=f"gact{j}",
                                    tag=f"gact{j}") for j in range(NGRP)]

                with ExitStack() as phase_a:
                    wpool = phase_a.enter_context(
                        tc.tile_pool(name="wp", bufs=1))

                    xf8 = xbpool.tile([128, NCC * TOK], F8E4, name="xf8",
                                      tag="xf8")
                    xf8b = xbpool.tile([128, NCC * TOK], F8E4, name="xf8b",
                                       tag="xf8b")
                    xbf = xbpool.tile([128, NCC * TOK], BF16, name="xbf",
                                      tag="xbf")
                    wt_all = wpool.tile([128, NCC * L], F8E4, name="wt_all",
                                        tag="wt")
                    wp_all = wpool.tile([128, NCC * L], BF16, name="wp_all",
                                        tag="wp")
                    wg_all = wpool.tile([128, NCC * L], F8E4, name="wg_all",
                                        tag="wg")

                    # input DMA schedule. There are TWO effective input
                    # pipes: the sync HWDGE queue, and a second HWDGE queue
                    # SHARED by the gpsimd and scalar engines (their
                    # descriptors interleave). Critical phase-1a tensors
                    # (wg halves, xf8 chunks, wt) ride the front of both
                    # pipes; phi inputs (wp/xbf) follow; xf8b (1c) and ww
                    # (phase 3) last.
                    CHW = NCC * 512          # flat cols per token chunk
                    HW = NCC * L // 2        # half of a weight tile
                    nc.sync.dma_start(xf8[:, 0:CHW], xf8_d[:, 0:CHW])
                    nc.gpsimd.dma_start(wg_all[:, 0:HW], wg_d[:, 0:HW])
                    nc.scalar.dma_start(xf8[:, ts(1, CHW)],
                                        xf8_d[:, ts(1, CHW)])
                    nc.sync.dma_start(wg_all[:, HW:2 * HW],
                                      wg_d[:, HW:2 * HW])
                    nc.gpsimd.dma_start(xf8[:, ts(2, CHW)],
                                        xf8_d[:, ts(2, CHW)])
                    nc.scalar.dma_start(xf8[:, ts(3, CHW)],
                                        xf8_d[:, ts(3, CHW)])
                    nc.sync.dma_start(wt_all[:], wt_d[:])
                    nc.gpsimd.dma_start(wp_all[:], wp_d[:])
                    nc.sync.dma_start(xbf[:, 0:CHW], xbf_d[:, 0:CHW])
                    nc.scalar.dma_start(xbf[:, ts(2, CHW)],
                                        xbf_d[:, ts(2, CHW)])
                    nc.sync.dma_start(xbf[:, ts(1, CHW)],
                                      xbf_d[:, ts(1, CHW)])
                    nc.gpsimd.dma_start(xbf[:, ts(3, CHW)],
                                        xbf_d[:, ts(3, CHW)])
                    nc.gpsimd.dma_start(mask_sb[:], mask_d[:])
                    nc.gpsimd.dma_start(gb_sb[:], gb_d[:])
                    nc.scalar.dma_start(xf8b[:], xf8b_d[:])
                    nc.sync.dma_start(ww_all[:], ww_d[:])
                    if USE_COLLECTIVE:
                        # warm-up collective #1: pays the CC stream's
                        # one-time setup during the DMA load
                        nc.gpsimd.dma_start(cc_warm_in[:], gb_d[:, 0:1])
                        nc.gpsimd.collective_compute(
                            "AllGather", OP.bypass,
                            replica_groups=[list(range(N_CORES))],
                            ins=[cc_warm_in.opt()], outs=[cc_warm_out.opt()])

                    # views (x tiles are token-chunk-major: (tck, a, k))
                    xv8 = xf8.rearrange("p (tk a k) -> p tk a k",
                                        tk=NTC, a=NCC)
                    # spatial: tok = j*T + t (channel-major copy)
                    xsp8 = xf8b.rearrange("p (a j t) -> p a t j",
                                          a=NCC, t=T)
                    xbv = xbf.rearrange("p (tk a k) -> p tk a k",
                                        tk=NTC, a=NCC)
                    wgv = wg_all.rearrange("p (a l) -> p a l", a=NCC)
                    wtv = wt_all.rearrange("p (a l) -> p a l", a=NCC)

                    def xsl(c, tck):
                        return xbv[:, tck, c, :]

                    def wsl(w, c, lc):
                        return w[:, c * L + lc * 128:c * L + (lc + 1) * 128]

                    # ------- phase 1a: g_act (xf8+wg) then theta (wt) ------
                    for jg in range(NGRP):
                        tck = jg // 4
                        ps = pbig.tile([128, 512], F32, name="ps_ga",
                                       tag="ps_big", bufs=4)
                        for cp in range(NCP):
                            nc.tensor.matmul(
                                ps[:],
                                xv8[:, tck, 2 * cp:2 * cp + 2,
                                    ts(jg - 4 * tck, 128)],
                                wgv[:, 2 * cp:2 * cp + 2, :],
                                start=(cp == 0), stop=(cp == NCP - 1),
                                perf_mode=DR)
                        nc.scalar.mul(g_act[jg][:], ps[:], G_DESCALE)

                    # theta: fp8 DoubleRow, lc-major with 4 concurrent tck
                    # PSUM groups -- each wt tile is loaded once, serving
                    # 4 back-to-back matmuls (consecutive loads dedupe)
                    for lc in range(NLC):
                        pss = [pbig.tile([128, 512], F32, name=f"ps_th{t}",
                                         tag="ps_big", bufs=4)
                               for t in range(NTC)]
                        for cp in range(NCP):
                            for tck in range(NTC):
                                nc.tensor.matmul(
                                    pss[tck][:],
                                    wtv[:, 2 * cp:2 * cp + 2, ts(lc, 128)],
                                    xv8[:, tck, 2 * cp:2 * cp + 2, :],
                                    start=(cp == 0), stop=(cp == NCP - 1),
                                    perf_mode=DR)
                        for tck in range(NTC):
                            dst = thT[:, lc * TOK + tck * 512:
                                      lc * TOK + tck * 512 + 512]
                            nc.vector.tensor_scalar_mul(dst, pss[tck][:],
                                                        G_DESCALE)

                    # ------- phase 1b: phi (bf16; wp + xbf) ---------------
                    # xbf is T-MAJOR (tq = t//4 quarter chunks) so the
                    # out-projection's residual add matches the contiguous
                    # t-major stT reads; phi's PSUM therefore comes out
                    # t-major and is scattered into the actor-major phT
                    # (stride-16 writes, alternating DVE/ACT -- both
                    # half-idle here, hidden under phi's 36us of matmul).
                    # lc-major over tq pairs (2 concurrent PSUM groups) so
                    # phi can start once xbf chunks 0-1 have landed.
                    phTv = phT.rearrange("p (a j t) -> p a t j",
                                         a=NLC, t=T)
                    for half in range(2):
                        tqs = (2 * half, 2 * half + 1)
                        for lc in range(NLC):
                            pss = [pbig.tile([128, 512], F32,
                                             name=f"ps_ph{t}",
                                             tag="ps_big", bufs=4)
                                   for t in tqs]
                            for c in range(NCC):
                                for k, tq in enumerate(tqs):
                                    nc.tensor.matmul(
                                        pss[k][:], wsl(wp_all, c, lc),
                                        xsl(c, tq),
                                        start=(c == 0), stop=(c == NCC - 1))
                            for k, tq in enumerate(tqs):
                                dst = phTv[:, lc, 4 * tq:4 * tq + 4, :]
                                src = pss[k].rearrange("p (t j) -> p t j",
                                                       t=4)
                                if lc % 2 == 0:
                                    nc.vector.tensor_copy(dst, src)
                                else:
                                    nc.scalar.copy(dst, src)

                    # ---- phase 1c: temporal attention + g_sp precompute --
                    # temporal INITIALIZES stT (t-major, strided write on the
                    # half-idle ACT engine); g_sp precomputed here so phase 2
                    # is only sw/sp
                    pend_tp = []   # (jg, twp)

                    def emit_tw(jg):
                        twp = psmall.tile([128, 128], F32, name="ps_tw",
                                          tag="ps_small", bufs=4)
                        for lc in range(NLC):
                            nc.tensor.matmul(
                                twp[:],
                                phT[:, lc * TOK + jg * 128:
                                    lc * TOK + jg * 128 + 128],
                                thT[:, lc * TOK + jg * 128:
                                    lc * TOK + jg * 128 + 128],
                                start=(lc == 0), stop=(lc == NLC - 1))
                        pend_tp.append((jg, twp))

                    def emit_tp():
                        jg, twp = pend_tp.pop(0)
                        sb = attnpool.tile([128, 128], F8E3, name="sb",
                                           tag="sb", bufs=3)
                        nc.vector.scalar_tensor_tensor(
                            out=sb[:], in0=twp[:], scalar=SB_SCALE,
                            in1=mask_sb[:], op0=OP.mult, op1=OP.mult)
                        pp = psmall.tile([128, 512], F32, name="ps_tp",
                                         tag="ps_small", bufs=4)
                        for lc in range(NLC):
                            nc.tensor.matmul(pp[:, ts(lc, 128)],
                                             g_act[jg][:, ts(lc, 128)], sb[:])
                        # pp free = (lc, j8, t16); stT t-major dst
                        # free = lc*2048 + t*128 + (8*jg + j)
                        dst = stT.rearrange("p (a t j) -> p a t j",
                                            a=NLC, t=T)[
                            :, :, :, ts(jg, JG)]
                        src = pp.rearrange("p (a j t) -> p a t j",
                                           a=NLC, j=JG)
                        nc.scalar.mul(dst, src, TP_SCALE)

                    def emit_gsp(i):
                        ps = pbig.tile([128, 512], F32, name="ps_g",
                                       tag="ps_big", bufs=4)
                        for cp in range(NCP):
                            nc.tensor.matmul(
                                ps[:],
                                xsp8[:, 2 * cp:2 * cp + 2, i:i + 1, :],
                                wgv[:, 2 * cp:2 * cp + 2, :],
                                start=(cp == 0), stop=(cp == NCP - 1),
                                perf_mode=DR)
                        nc.scalar.mul(g_sp[i][:], ps[:], G_DESCALE)

                    first_done = False
                    for k in range(NGRP):
                        emit_gsp(k)
                        emit_tw(k)
                        if not first_done and USE_COLLECTIVE:
                            # warm-up collective #2, keyed on 1c data so the
                            # CC stream is recently-used when the real stats
                            # collective triggers
                            nc.gpsimd.dma_start(cc_w2_in[:],
                                                g_sp[0][:, 0:1])
                            nc.gpsimd.collective_compute(
                                "AllGather", OP.bypass,
                                replica_groups=[list(range(N_CORES))],
                                ins=[cc_w2_in.opt()], outs=[cc_w2_out.opt()])
                            first_done = True
                        if len(pend_tp) >= 2:
                            emit_tp()
                    while pend_tp:
                        emit_tp()

                    # ------- phase 2: spatial attention (ADD into stT) ----
                    pend_sp = []   # (i, swp)

                    def sp_view(tile_ap, i):
                        return tile_ap.rearrange(
                            "p (j t) -> p t j", t=T)[:, i:i + 1, :]

                    def thsl(tt, lc):
                        return tt[:, lc * TOK:(lc + 1) * TOK]

                    def emit_sw(i):
                        swp = psmall.tile([128, 128], F32, name="ps_sw",
                                          tag="ps_small", bufs=4)
                        for lc in range(NLC):
                            nc.tensor.matmul(swp[:],
                                             sp_view(thsl(phT, lc), i),
                                             sp_view(thsl(thT, lc), i),
                                             start=(lc == 0),
                                             stop=(lc == NLC - 1))
                        pend_sp.append((i, swp))

                    def emit_sp():
                        i, swp = pend_sp.pop(0)
                        swb = attnpool.tile([128, 128], F8E3, name="swb",
                                            tag="swb", bufs=3)
                        nc.scalar.mul(swb[:], swp[:], SB_SCALE)
                        pp = psmall.tile([128, 512], F32, name="ps_sp",
                                         tag="ps_small", bufs=4)
                        for lc in range(NLC):
                            nc.tensor.matmul(pp[:, ts(lc, 128)],
                                             g_sp[i][:, ts(lc, 128)], swb[:])
                        # t-major stT: the t=i row (all 128 actors) is a
                        # contiguous 128-run per lc chunk -> fast DVE RMW
                        dst = stT.rearrange("p (a t j) -> p a t j",
                                            a=NLC, t=T)[:, :, i, :]
                        src = pp.rearrange("p (a j) -> p a j", a=NLC)
                        nc.vector.scalar_tensor_tensor(
                            out=dst, in0=src, scalar=SP_SCALE, in1=dst,
                            op0=OP.mult, op1=OP.add)

                    for i in range(T):
                        emit_sw(i)
                        if len(pend_sp) >= 2:
                            emit_sp()
                    while pend_sp:
                        emit_sp()

            # ------- phase 3: out-projection + residual + stats -------
            with tc.tile_pool(name="outp", bufs=1) as outpool, \
                 tc.tile_pool(name="yp", bufs=1) as ypool, \
                 tc.tile_pool(name="sqp", bufs=1) as sqpool:
                out_sb = []
                inv_n = 1.0 / float(NTOK_GLOBAL)

                def emit_outproj(ct):
                    o = outpool.tile([128, TOK], BF16, name=f"out{ct}",
                                     tag=f"out{ct}")
                    out_sb.append(o)
                    # tq-inner with 4 concurrent PSUM groups: each ww
                    # weight tile serves 4 back-to-back matmuls; the moving
                    # operand is a CONTIGUOUS 512-col t-major stT slice
                    # (tokens t in [4tq, 4tq+4), all actors), matching the
                    # t-major xbf/output layout
                    pss = [pbig.tile([128, 512], F32, name=f"ps_out{t}",
                                     tag="ps_big", bufs=4)
                           for t in range(NTC)]
                    for lc in range(NLC):
                        for tq in range(NTC):
                            nc.tensor.matmul(
                                pss[tq][:],
                                ww_all[:, lc * C + ct * 128:
                                       lc * C + (ct + 1) * 128],
                                stT[:, lc * TOK + tq * 512:
                                    lc * TOK + tq * 512 + 512],
                                start=(lc == 0), stop=(lc == NLC - 1))
                    for tq in range(NTC):
                        col = ct * NTC + tq
                        nc.vector.scalar_tensor_tensor(
                            out=o[:, ts(tq, 512)], in0=pss[tq][:],
                            scalar=OUT_DESCALE,
                            in1=xbf.rearrange("p (tk a k) -> p tk a k",
                                              tk=NTC, a=NCC)[:, tq, ct, :],
                            op0=OP.mult, op1=OP.add,
                            accum_out=stat_sum[:, col:col + 1])
                    # one whole-ct square (vs per-tq): 1/4 the ACT
                    # read-accumulator ops on the stats critical path
                    sq = sqpool.tile([128, TOK], F32, name="sqscr",
                                     tag="sq", bufs=2)
                    nc.scalar.activation(
                        sq[:], o[:], ACT_FN.Square,
                        accum_out=stat_sq[:, ct:ct + 1])

                # stats are collected and all-gathered in TWO ct-halves:
                # the first AllGather (channels 0-511) fires as soon as
                # out-projection chunks 0-3 land and completes under the
                # remaining chunks' compute; only the second (tiny) AG's
                # ~5us latency is exposed, and the first half's BN params +
                # applies + stores overlap it.
                cc_h_in = [drampool.tile([128, 8], F32, name=f"cc_in{g}",
                                         tag=f"cc_in{g}") for g in range(2)]
                cc_h_out = [drampool.tile([N_CORES * 128, 8], F32,
                                          name=f"cc_out{g}",
                                          tag=f"cc_out{g}") for g in range(2)]

                def emit_stats_trigger(g):
                    """Reduce + bounce + AllGather trigger for ct in
                    [4g, 4g+4). No completion-gated work here, so the
                    second trigger is not stuck behind the first
                    AllGather's readback in the gpsimd FIFO."""
                    nc.vector.tensor_reduce(
                        red_in[:, 8 * g:8 * g + 4],
                        stat_sum.rearrange("p (a b) -> p a b",
                                           a=8)[:, 4 * g:4 * g + 4, :],
                        axis=AX.X, op=OP.add)
                    nc.vector.tensor_copy(red_in[:, 8 * g + 4:8 * g + 8],
                                          stat_sq[:, 4 * g:4 * g + 4])
                    if USE_COLLECTIVE:
                        nc.gpsimd.dma_start(cc_h_in[g][:],
                                            red_in[:, 8 * g:8 * g + 8])
                        nc.gpsimd.collective_compute(
                            "AllGather", OP.bypass,
                            replica_groups=[list(range(N_CORES))],
                            ins=[cc_h_in[g].opt()],
                            outs=[cc_h_out[g].opt()])

                def emit_stats_read(g):
                    """Readback + cross-core reduce for half g."""
                    if USE_COLLECTIVE:
                        src = cc_h_out[g].rearrange("(r p) c -> p r c",
                                                    r=N_CORES)
                        dst = ag_sb.rearrange("p (g r c) -> p g r c",
                                              g=2, r=N_CORES)[:, g]
                        nc.gpsimd.dma_start(dst, src)
                        agv = ag_sb.rearrange("p (g r c) -> p g c r",
                                              g=2, r=N_CORES)
                        # cols 0:4 are sums, 4:8 sumsq for this half
                        nc.vector.tensor_reduce(
                            red_out[:, 4 * g:4 * g + 4],
                            agv[:, g, 0:4, :], axis=AX.X, op=OP.add)
                        nc.vector.tensor_reduce(
                            red_out[:, 8 + 4 * g:8 + 4 * g + 4],
                            agv[:, g, 4:8, :], axis=AX.X, op=OP.add)
                    else:
                        nc.vector.tensor_scalar_mul(
                            red_out[:, 4 * g:4 * g + 4],
                            red_in[:, 8 * g:8 * g + 4], float(N_CORES))
                        nc.vector.tensor_scalar_mul(
                            red_out[:, 8 + 4 * g:8 + 4 * g + 4],
                            red_in[:, 8 * g + 4:8 * g + 8], float(N_CORES))

                def emit_bn_params(g):
                    lo, hi = 4 * g, 4 * g + 4
                    mean = statpool.tile([128, 4], F32, name=f"mean{g}",
                                         tag=f"mean{g}")
                    var = statpool.tile([128, 4], F32, name=f"var{g}",
                                        tag=f"var{g}")
                    std = statpool.tile([128, 4], F32, name=f"std{g}",
                                        tag=f"std{g}")
                    rstd = statpool.tile([128, 4], F32, name=f"rstd{g}",
                                         tag=f"rstd{g}")
                    nc.vector.tensor_scalar_mul(mean[:], red_out[:, lo:hi],
                                                inv_n)
                    nc.vector.tensor_scalar_mul(var[:],
                                                red_out[:, 8 + lo:8 + hi],
                                                inv_n)
                    nc.vector.tensor_mul(std[:], mean[:], mean[:])
                    nc.vector.tensor_tensor(var[:], var[:], std[:],
                                            op=OP.subtract)
                    nc.vector.tensor_scalar_add(var[:], var[:], BN_EPS)
                    nc.scalar.activation(std[:], var[:], ACT_FN.Sqrt, bias=0.0)
                    nc.vector.reciprocal(rstd[:], std[:])
                    nc.vector.tensor_mul(scalev[:, lo:hi], rstd[:],
                                         gb_sb[:, lo:hi])
                    nc.vector.tensor_mul(rstd[:], mean[:], scalev[:, lo:hi])
                    nc.vector.tensor_tensor(biasv[:, lo:hi],
                                            gb_sb[:, 8 + lo:8 + hi], rstd[:],
                                            op=OP.subtract)

                def emit_apply(ct):
                    # DVE is ~2.3x faster per op here than ACT: give DVE 13
                    # of 16 half-tiles, ACT 3. First-half stores use all
                    # three rings; second-half stores avoid gpsimd so its
                    # end-of-kernel queue drain overlaps the second
                    # AllGather instead of trailing it.
                    for h in range(2):
                        i = 2 * ct + h
                        src = out_sb[ct][:, ts(h, 1024)]
                        if i % 5 == 4:
                            y = ypool.tile([128, 1024], BF16, name="ya",
                                           tag="ya", bufs=4)
                            nc.scalar.activation(
                                y[:], src, ACT_FN.Identity,
                                scale=scalev[:, ct:ct + 1],
                                bias=biasv[:, ct:ct + 1])
                        else:
                            y = ypool.tile([128, 1024], BF16, name="yb",
                                           tag="yb", bufs=8)
                            nc.vector.tensor_scalar(
                                out=y[:], in0=src,
                                scalar1=scalev[:, ct:ct + 1],
                                scalar2=biasv[:, ct:ct + 1],
                                op0=OP.mult, op1=OP.add)
                        if i < 8:
                            ring = (nc.sync, nc.gpsimd, nc.scalar)[i % 3]
                        else:
                            ring = (nc.sync, nc.scalar)[i % 2]
                        ring.dma_start(outy_d[ts(ct, 128), ts(h, 1024)],
                                       y[:])

                for ct in range(4):
                    emit_outproj(ct)
                emit_stats_trigger(0)
                for ct in range(4, NCC):
                    emit_outproj(ct)
                emit_stats_trigger(1)
                emit_stats_read(0)
                emit_bn_params(0)
                for ct in range(4):
                    emit_apply(ct)
                emit_stats_read(1)
                emit_bn_params(1)
                for ct in range(4, NCC):
                    emit_apply(ct)

    nc.compile()
    return nc


def _get_compiled():
    global _compiled
    if _compiled is None:
        _compiled = _build()
    return _compiled


def kernel(x, Wt, Wp, Wg, Ww, gamma, beta, _trace=False, _trace_kwargs=None):
    global _last_results
    nc = _get_compiled()

    x = np.asarray(x, dtype=np.float32)
    Wt = np.asarray(Wt, dtype=np.float32)
    Wp = np.asarray(Wp, dtype=np.float32)
    Wg = np.asarray(Wg, dtype=np.float32)
    Ww = np.asarray(Ww, dtype=np.float32)
    gamma = np.asarray(gamma, dtype=np.float32)
    beta = np.asarray(beta, dtype=np.float32)

    bf = ml_dtypes.bfloat16
    f8e4 = ml_dtypes.float8_e4m3
    f8e3 = ml_dtypes.float8_e3m4

    def shuf(a):
        """[n*128, F] -> tile layout [128, n*F] (chunk-major free axis)."""
        n = a.shape[0] // 128
        return np.ascontiguousarray(
            a.reshape(n, 128, a.shape[1]).transpose(1, 0, 2).reshape(128, -1))

    def shuf_x(a):
        """[C, TOK] -> token-chunk-major tile layout [128, (tck, a, 512)]."""
        return np.ascontiguousarray(
            a.reshape(NCC, 128, NTC, 512).transpose(1, 2, 0, 3)
            .reshape(128, -1))

    wt_t = shuf((Wt.T * WG_SCALE).astype(f8e4))       # [C, L] -> tile
    wp_t = shuf(Wp.T.astype(bf))
    wg_t = shuf((Wg.T * WG_SCALE).astype(f8e4))
    ww_t = shuf((Ww.T * WW_SCALE).astype(f8e3))       # [L, C] -> tile
    r = np.arange(128)
    mask = (r[:, None] // T == r[None, :] // T).astype(bf)
    gb = np.concatenate(
        [gamma.reshape(NCC, 128).T,
         beta.reshape(NCC, 128).T], axis=1).astype(np.float32)  # [128, 16]

    # xf8/xf8b: actor-major token order (tok = j*T + t);
    # xbf: T-MAJOR token order (tok = t*N + j) to match the t-major stT
    # reads in the out-projection
    xa = x.transpose(0, 2, 1, 3).reshape(B, TOK, C)
    xt = x.reshape(B, TOK, C)                          # [B, (t n), C]
    in_maps = []
    for b in range(B):
        xT = np.ascontiguousarray(xa[b].T)            # [C, TOK] f32
        x8 = xT.astype(f8e4)
        xTt = np.ascontiguousarray(xt[b].T)           # [C, (t n)] f32
        in_maps.append(dict(
            xf8=shuf_x(x8), xf8b=shuf(x8), xbf=shuf_x(xTt.astype(bf)),
            wt=wt_t, wp=wp_t, wg=wg_t, ww=ww_t,
            mask=mask, gb=gb))

    res = run_bass_kernel_spmd(nc, in_maps, list(range(N_CORES)),
                               trace=_trace, **(_trace_kwargs or {}))
    _last_results = res

    ys = []
    for b in range(B):
        # outy cols are t-major: tok = t*N + j
        o = np.asarray(res.results[b]["outy"], dtype=np.float32)   # [C, TOK]
        ys.append(o.T.reshape(T, N, C))
    return np.stack(ys)
